# revision 4
# baseline (speedup 1.0000x reference)
"""Trainium2 Bass kernel for nn_MultiHeadAttention_833223655722.

Data-parallel over batch (16 / 8 cores = 2 per core). All matmuls bf16 with
fp32 PSUM accumulation. LayerNorm mean folded into centered projection
weights; rstd factors folded into the scores consume (fast path).

v2: one-level Strassen on all four E x E matmuls (7/8 multiply count):
  - Q/K projections (weight-stationary, qT[f,t] output): token split halves
    the moving free dim -> N=256 matmuls (still full rate, LDW hidden).
  - V projection and out-projection run the same scheme "executed
    transposed" (data-stationary, moving weight combos) so the weight
    output dim stays on the free axis -> N=512 matmuls.
  - The seven x-side operands (host-precomputed, bf16) serve Q/K as moving
    operands and V as stationary operands (identical storage layout).
  - W-side combos precomputed on host; C-block assembly on DVE from PSUM.
LN/L2 stats: ACT square + DVE pair-sum tree + a single ones-matmul
(1 matmul per stat instead of 4).

The general path (nonzero biases / non-unit gains) keeps the v1 direct
implementation.
"""

import sys
import types

import numpy as np
import ml_dtypes

import concourse.bass as bass
import concourse.mybir as mybir
import concourse.tile as tile
from concourse import bacc, library_config
from concourse import bass_utils
from concourse.bass_utils import run_bass_kernel_spmd

# ---------------------------------------------------------------- constants
B, S, E, H = 16, 512, 4096, 8
HD = E // H            # 512 (== S)
N_CORES = 8
NB = B // N_CORES      # 2 batches per core
P = 128
KO = E // P            # 32 contraction chunks over E
KO2 = KO // 2          # 16 chunks per e-half
TC = S // P            # 4 token chunks
DC = HD // P           # 4 head-dim chunks
FC = 2 * HD // P       # 8 GeGLU chunks
NGB = E // 512         # 8 out-proj column blocks
EH = E // 2            # 2048
SH = S // 2            # 256
LN_EPS = 1e-5
NORM_EPS = 1e-12

F32 = mybir.dt.float32
BF16 = mybir.dt.bfloat16
BF = ml_dtypes.bfloat16
AF = mybir.ActivationFunctionType
ALU = mybir.AluOpType


def _install_ntff_hook():
    """Register the NTFF profile hook missing from this image's antenv."""
    try:
        import antenv
        from trn_agent_boot.trn_boot import _ntff_profile_via_ctypes

        if "antenv.axon_hooks" in sys.modules:
            return
        hook = _ntff_profile_via_ctypes("/opt/axon/libaxon_pjrt.so")
        mod = types.ModuleType("antenv.axon_hooks")
        mod.get_axon_ntff_profile_hook = lambda: hook
        mod.set_axon_ntff_profile_hook = lambda h: None
        sys.modules["antenv.axon_hooks"] = mod
        antenv.axon_hooks = mod
        bass_utils.upload_artifacts = lambda tmpdir: tmpdir
    except Exception:
        pass


# =====================================================================
# fast path (all biases zero, gains one): Strassen build
# =====================================================================

def _build_fast_program():
    nc = bacc.Bacc("TRN2", target_bir_lowering=False, debug=False, num_devices=N_CORES)

    def dm(name, shape, dt, **kw):
        return nc.dram_tensor(name, shape, dt, **kw).ap()

    # seven x-side operands per batch, layout [op, ko(e), P(e), t(256)]
    xs_d = dm("xs", [NB, 7, KO2, P, SH], BF16, kind="ExternalInput")
    # W-side strassen operands, layout [op, ko(e), P(e), out(2048)]
    wqst_d = dm("wqst", [7, KO2, P, EH], BF16, kind="ExternalInput")
    wkst_d = dm("wkst", [7, KO2, P, EH], BF16, kind="ExternalInput")
    wvst_d = dm("wvst", [7, KO2, P, EH], BF16, kind="ExternalInput")
    wost_d = dm("wost", [7, KO2, P, EH], BF16, kind="ExternalInput")
    wgt_d = dm("wgt", [TC, P, 2 * HD], BF16, kind="ExternalInput")
    y_d = dm("y", [NB, S, E], F32, kind="ExternalOutput")
    rksc_d = dm("rksc", [NB * H, 512], F32)

    with tile.TileContext(nc) as tc:
        with (
            tc.tile_pool(name="singles", bufs=1) as singles,
            tc.tile_pool(name="obtp", bufs=1) as obtp,
            tc.tile_pool(name="rows", bufs=4) as rowsp,
            tc.tile_pool(name="bc", bufs=3) as bcp,
            tc.tile_pool(name="cols", bufs=4) as colsp,
            tc.tile_pool(name="ps", bufs=4, space="PSUM") as psp,
        ):
            nc.gpsimd.load_library(library_config.attn)

            ones_col = singles.tile([P, 1], BF16)
            nc.vector.memset(ones_col[:], 1.0)
            eps_qf = singles.tile([1, 1], F32)
            nc.vector.memset(eps_qf[:], float(HD * LN_EPS))
            eps_ln = singles.tile([1, 1], F32)
            nc.vector.memset(eps_ln[:], float(LN_EPS))
            eps_n2 = singles.tile([1, 1], F32)
            nc.vector.memset(eps_n2[:], float(NORM_EPS**2))
            wgt_sb = singles.tile([P, TC, 2 * HD], BF16)
            nc.sync.dma_start(wgt_sb[:], wgt_d.rearrange("t p f -> p t f"))

            _ctr = [0]

            def punit():
                _ctr[0] += 1
                return psp.tile([P, 2, 512], F32, tag="u", name=f"u{_ctr[0]}")

            def row(name):
                _ctr[0] += 1
                return rowsp.tile([1, 512], F32, tag="row", name=f"{name}{_ctr[0]}")

            def bcast128(row_ap, name):
                _ctr[0] += 1
                t = bcp.tile([P, 512], F32, tag="bc", name=f"{name}{_ctr[0]}")
                nc.gpsimd.partition_broadcast(t[:], row_ap)
                return t

            for b in range(NB):
                obt = obtp.tile([P, KO, S], BF16, tag="obt", name=f"obt{b}")

                with (
                    tc.tile_pool(name=f"xsp{b}", bufs=1) as xsp,
                    tc.tile_pool(name=f"wsl{b}", bufs=3) as wslp,
                    tc.tile_pool(name=f"wvsl{b}", bufs=2) as wvslp,
                    tc.tile_pool(name=f"act{b}", bufs=2) as actp,
                    tc.tile_pool(name=f"act3{b}", bufs=2) as actp3,
                    tc.tile_pool(name=f"sqp{b}", bufs=1) as sqp,
                    tc.tile_pool(name=f"ctmp{b}", bufs=1) as ctmpp,
                    tc.tile_pool(name=f"vtmp{b}", bufs=1) as vtmpp,
                    tc.tile_pool(name=f"stt{b}", bufs=2) as sttp,
                ):
                    xs = xsp.tile([P, 7, KO2, SH], BF16, tag="xs", name=f"xs{b}")
                    for op in range(7):
                        nc.gpsimd.dma_start(
                            xs[:, op],
                            xs_d[b, op].rearrange("k p t -> p k t"),
                        )

                    def wslab_qk(w_dram, op, mc):
                        _ctr[0] += 1
                        t = wslp.tile([P, KO2, P], BF16, tag="wsl", name=f"w{_ctr[0]}")
                        nc.sync.dma_start(
                            t[:],
                            w_dram[op, :, :, mc * P : (mc + 1) * P].rearrange(
                                "k p f -> p k f"
                            ),
                        )
                        return t

                    def wslab_mov(w_dram, op, half, col0):
                        """[P, 8, 512] slab: ko-half of a moving W operand."""
                        _ctr[0] += 1
                        t = wvslp.tile([P, KO2 // 2, 512], BF16, tag="wvsl", name=f"wv{_ctr[0]}")
                        nc.sync.dma_start(
                            t[:],
                            w_dram[op, half * 8 : half * 8 + 8, :, col0 : col0 + 512]
                            .rearrange("k p f -> p k f"),
                        )
                        return t

                    def stat_pre(src, name):
                        """src [P,4,512] bf16 -> ACT square + DVE tree -> [P,512] bf16."""
                        _ctr[0] += 1
                        sq = sqp.tile([P, DC, S], BF16, tag="sq", name=f"sq{name}{_ctr[0]}")
                        nc.scalar.activation(sq[:], src[:], AF.Square)
                        t0 = sttp.tile([P, 512], F32, tag="st0", name=f"st0{_ctr[0]}", bufs=1)
                        t1 = sttp.tile([P, 512], F32, tag="st1", name=f"st1{_ctr[0]}", bufs=1)
                        sbf = sttp.tile([P, 512], BF16, tag="stb", name=f"stb{_ctr[0]}")
                        nc.vector.tensor_tensor(t0[:], sq[:, 0, :], sq[:, 1, :], ALU.add)
                        nc.vector.tensor_tensor(t1[:], sq[:, 2, :], sq[:, 3, :], ALU.add)
                        nc.vector.tensor_tensor(sbf[:], t0[:], t1[:], ALU.add)
                        return sbf

                    def stat_mm(sbf, stat_slice):
                        nc.tensor.matmul(stat_slice, ones_col[:], sbf[:], start=True, stop=True)

                    # C-combo helper for Q/K quarters (one PSUM read per op)
                    def qk_combine(quarters, out_lo, out_hi, dc):
                        """quarters: list of 7 psum APs [P, 256] (M1..M7)."""
                        M = quarters
                        t = ctmpp.tile([P, SH], F32, tag="ct", name=f"ct{_ctr[0]}a")
                        _ctr[0] += 1
                        nc.vector.tensor_copy(t[:], M[0])
                        nc.vector.tensor_tensor(t[:], t[:], M[3], ALU.add)
                        nc.vector.tensor_tensor(t[:], t[:], M[4], ALU.subtract)
                        nc.vector.tensor_tensor(out_lo[:, dc, 0:SH], t[:], M[6], ALU.add)
                        t2 = ctmpp.tile([P, SH], F32, tag="ct2", name=f"ct{_ctr[0]}b")
                        _ctr[0] += 1
                        nc.vector.tensor_copy(t2[:], M[2])
                        nc.vector.tensor_tensor(out_lo[:, dc, SH:S], t2[:], M[4], ALU.add)
                        t3 = ctmpp.tile([P, SH], F32, tag="ct3", name=f"ct{_ctr[0]}c")
                        _ctr[0] += 1
                        nc.vector.tensor_copy(t3[:], M[1])
                        nc.vector.tensor_tensor(out_hi[:, dc, 0:SH], t3[:], M[3], ALU.add)
                        t4 = ctmpp.tile([P, SH], F32, tag="ct4", name=f"ct{_ctr[0]}d")
                        _ctr[0] += 1
                        nc.vector.tensor_copy(t4[:], M[0])
                        nc.vector.tensor_tensor(t4[:], t4[:], M[1], ALU.subtract)
                        nc.vector.tensor_tensor(t4[:], t4[:], M[2], ALU.add)
                        nc.vector.tensor_tensor(out_hi[:, dc, SH:S], t4[:], M[5], ALU.add)

                    def qk_proj(w_dram, g, out_lo, out_hi, name, inject=None):
                        """Strassen products for f-rows of heads (g, 4+g)."""
                        for mc in range(4 * g, 4 * g + 4):
                            if inject and (mc - 4 * g) in inject:
                                inject[mc - 4 * g]()
                            ua = punit()
                            ub = punit()
                            quarters = [
                                ua[:, 0, 0:SH], ua[:, 0, SH:512],
                                ua[:, 1, 0:SH], ua[:, 1, SH:512],
                                ub[:, 0, 0:SH], ub[:, 0, SH:512],
                                ub[:, 1, 0:SH],
                            ]
                            for op in range(7):
                                slab = wslab_qk(w_dram, op, mc)
                                for ko in range(KO2):
                                    nc.tensor.matmul(
                                        quarters[op],
                                        slab[:, ko, :],
                                        xs[:, op, ko, :],
                                        start=(ko == 0),
                                        stop=(ko == KO2 - 1),
                                    )
                            qk_combine(quarters, out_lo, out_hi, mc - 4 * g)

                    def v_proj(g, vc_lo, vc_hi, inject=None):
                        """Transposed-scheme products, d-columns of heads (g, 4+g)."""
                        col0 = g * 512
                        t11 = vtmpp.tile([P, 2, 512], F32, tag="v11", name=f"v11{b}{g}")
                        t12 = vtmpp.tile([P, 2, 512], BF16, tag="v12", name=f"v12{b}{g}")
                        t21 = vtmpp.tile([P, 2, 512], BF16, tag="v21", name=f"v21{b}{g}")
                        t22 = vtmpp.tile([P, 2, 512], F32, tag="v22", name=f"v22{b}{g}")
                        for op in range(7):
                            if inject and op in inject:
                                inject[op]()
                            u = punit()
                            for half in range(2):
                                slab = wslab_mov(wvst_d, op, half, col0)
                                for kk in range(KO2 // 2):
                                    ko = half * 8 + kk
                                    for tcc in range(2):
                                        nc.tensor.matmul(
                                            u[:, tcc, :],
                                            xs[:, op, ko, tcc * P : (tcc + 1) * P],
                                            slab[:, kk, :],
                                            start=(ko == 0),
                                            stop=(ko == KO2 - 1),
                                        )
                            # greedy C accumulation (P1..P7 = op 0..6)
                            if op == 0:
                                nc.vector.tensor_copy(t11[:], u[:])
                                nc.vector.tensor_copy(t22[:], u[:])
                            elif op == 1:
                                nc.vector.tensor_copy(t12[:], u[:])
                                nc.vector.tensor_tensor(t22[:], t22[:], u[:], ALU.subtract)
                            elif op == 2:
                                nc.vector.tensor_copy(t21[:], u[:])
                                nc.vector.tensor_tensor(t22[:], t22[:], u[:], ALU.add)
                            elif op == 3:
                                nc.vector.tensor_tensor(t11[:], t11[:], u[:], ALU.add)
                                nc.vector.tensor_tensor(vc_hi[:, 0:2, :], t12[:], u[:], ALU.add)
                            elif op == 4:
                                nc.vector.tensor_tensor(t11[:], t11[:], u[:], ALU.subtract)
                                nc.vector.tensor_tensor(vc_lo[:, 2:4, :], t21[:], u[:], ALU.add)
                            elif op == 5:
                                nc.vector.tensor_tensor(vc_hi[:, 2:4, :], t22[:], u[:], ALU.add)
                            else:
                                nc.vector.tensor_tensor(vc_lo[:, 0:2, :], t11[:], u[:], ALU.add)

                    # ---------------- attention for one head ----------------
                    def attention(h, qc, kc, vc, rqb, rk_cols):
                        sunits = [punit(), punit()]
                        for t_ in range(TC):
                            for dc in range(DC):
                                nc.tensor.matmul(
                                    sunits[t_ // 2][:, t_ % 2, :],
                                    kc[:, dc, t_ * P : (t_ + 1) * P],
                                    qc[:, dc, :],
                                    start=(dc == 0),
                                    stop=(dc == DC - 1),
                                )
                        sc = actp3.tile([P, TC, S], BF16, tag="sc", name=f"sc{h}{b}")
                        for t_ in range(TC):
                            nc.vector.scalar_tensor_tensor(
                                sc[:, t_, :],
                                sunits[t_ // 2][:, t_ % 2, :],
                                rk_cols[:, t_ : t_ + 1],
                                rqb[:],
                                ALU.mult,
                                ALU.mult,
                            )
                        gunits = [punit(), punit()]
                        for i in range(DC):
                            fc = DC + i
                            for t_ in range(TC):
                                nc.tensor.matmul(
                                    gunits[i // 2][:, i % 2, :],
                                    wgt_sb[:, t_, fc * P : (fc + 1) * P],
                                    sc[:, t_, :],
                                    start=(t_ == 0),
                                    stop=(t_ == TC - 1),
                                )
                        vunits2 = [punit(), punit()]
                        for i in range(DC):
                            for t_ in range(TC):
                                nc.tensor.matmul(
                                    vunits2[i // 2][:, i % 2, :],
                                    wgt_sb[:, t_, i * P : (i + 1) * P],
                                    sc[:, t_, :],
                                    start=(t_ == 0),
                                    stop=(t_ == TC - 1),
                                )
                        gel = actp3.tile([P, DC, S], BF16, tag="gel", name=f"gel{h}{b}", bufs=1)
                        for i in range(DC):
                            nc.scalar.activation(
                                gel[:, i, :],
                                gunits[i // 2][:, i % 2, :],
                                AF.Gelu,
                                bias=0.0,
                            )
                        wv = actp3.tile([P, DC, S], BF16, tag="wv", name=f"wv{h}{b}", bufs=1)
                        for u in range(2):
                            nc.vector.tensor_copy(
                                wv[:, 2 * u : 2 * u + 2, :], vunits2[u][:]
                            )
                            nc.vector.tensor_mul(
                                wv[:, 2 * u : 2 * u + 2, :],
                                wv[:, 2 * u : 2 * u + 2, :],
                                gel[:, 2 * u : 2 * u + 2, :],
                            )
                        # L2 stats: tree now, matmul after the out MMs
                        sbf_w = stat_pre(wv, f"w{h}")
                        stat2 = psp.tile([1, 2, 512], F32, tag="u", name=f"st2{h}{b}")
                        # out matmuls (t-major)
                        ounits = [punit(), punit()]
                        for t_ in range(TC):
                            for dc in range(DC):
                                nc.tensor.matmul(
                                    ounits[dc // 2][:, dc % 2, :],
                                    vc[:, t_, dc * P : (dc + 1) * P],
                                    wv[:, t_, :],
                                    start=(t_ == 0),
                                    stop=(t_ == TC - 1),
                                )
                        stat_mm(sbf_w, stat2[0:1, 0, :])
                        nrow = row("nr")
                        nc.scalar.activation(
                            nrow[:], stat2[0:1, 0, :], AF.Sqrt, bias=eps_n2[:]
                        )
                        rr = row("rr")
                        nc.vector.reciprocal_approx_fast(rr[:], nrow[:])
                        rb = bcast128(rr[:], "rb")
                        for u in range(2):
                            nc.vector.tensor_tensor(
                                obt[:, h * DC + 2 * u : h * DC + 2 * u + 2, :],
                                ounits[u][:],
                                rb[:, None, :].to_broadcast((P, 2, 512)),
                                ALU.mult,
                            )

                    # =============== head-pair groups ===============
                    for g in range(4):
                        hl, hh = g, 4 + g

                        qc_lo = actp.tile([P, DC, S], BF16, tag="qc", name=f"qc{hl}{b}")
                        qc_hi = actp.tile([P, DC, S], BF16, tag="qc", name=f"qc{hh}{b}")
                        qk_proj(wqst_d, g, qc_lo, qc_hi, "q")

                        kc_lo = actp.tile([P, DC, S], BF16, tag="kc", name=f"kc{hl}{b}")
                        kc_hi = actp.tile([P, DC, S], BF16, tag="kc", name=f"kc{hh}{b}")
                        # q stats + rows interleaved into the K product stream
                        sbf_ql = stat_pre(qc_lo, f"ql{g}")
                        sbf_qh = stat_pre(qc_hi, f"qh{g}")
                        stat = psp.tile([1, 2, 512], F32, tag="u", name=f"stq{g}{b}")
                        rqbs = []

                        def q_stats_mm():
                            stat_mm(sbf_ql, stat[0:1, 0, :])
                            stat_mm(sbf_qh, stat[0:1, 1, :])

                        def q_rows():
                            for slot in range(2):
                                sd_q = row("sdq")
                                nc.scalar.activation(
                                    sd_q[:], stat[0:1, slot, :], AF.Sqrt, bias=eps_qf[:]
                                )
                                rq_row = row("rq")
                                nc.vector.reciprocal_approx_fast(rq_row[:], sd_q[:])
                                rqbs.append(bcast128(rq_row[:], "rqb"))

                        qk_proj(wkst_d, g, kc_lo, kc_hi, "k",
                                inject={1: q_stats_mm, 2: q_rows})

                        vc_lo = actp.tile([P, TC, HD], BF16, tag="vc", name=f"vc{hl}{b}")
                        vc_hi = actp.tile([P, TC, HD], BF16, tag="vc", name=f"vc{hh}{b}")
                        # k stats + rows interleaved into the V product stream
                        sbf_kl = stat_pre(kc_lo, f"kl{g}")
                        sbf_kh = stat_pre(kc_hi, f"kh{g}")
                        statk = psp.tile([1, 2, 512], F32, tag="u", name=f"stk{g}{b}")
                        rkcs = []

                        def k_stats_mm():
                            stat_mm(sbf_kl, statk[0:1, 0, :])
                            stat_mm(sbf_kh, statk[0:1, 1, :])

                        def k_rows():
                            for slot, h in ((0, hl), (1, hh)):
                                sd_k = row("sdk")
                                nc.scalar.activation(
                                    sd_k[:], statk[0:1, slot, :], AF.Sqrt,
                                    bias=eps_ln[:], scale=float(1.0 / HD),
                                )
                                idx = b * H + h
                                nc.sync.dma_start(rksc_d[idx : idx + 1, :], sd_k[:])
                                sd_cols = colsp.tile([P, TC], F32, tag="cols", name=f"sdc{h}{b}")
                                nc.sync.dma_start(
                                    sd_cols[:], rksc_d[idx].rearrange("(c p) -> p c", p=P)
                                )
                                rk_cols = colsp.tile([P, TC], F32, tag="cols", name=f"rkc{h}{b}")
                                nc.vector.reciprocal_approx_fast(rk_cols[:], sd_cols[:])
                                rkcs.append(rk_cols)

                        v_proj(g, vc_lo, vc_hi,
                               inject={2: k_stats_mm, 4: k_rows})

                        attention(hl, qc_lo, kc_lo, vc_lo, rqbs[0], rkcs[0])
                        attention(hh, qc_hi, kc_hi, vc_hi, rqbs[1], rkcs[1])

                # ---------------- out-projection (Strassen) ----------------
                with (
                    tc.tile_pool(name=f"ocp{b}", bufs=1) as ocp,
                    tc.tile_pool(name=f"wosl{b}", bufs=3) as woslp,
                    tc.tile_pool(name=f"yac{b}", bufs=1) as yacp,
                ):
                    # obtT blocks: O11=obt[:,0:16,0:256] O12=[...,256:512]
                    #              O21=obt[:,16:32,0:256] O22=[...,16:32,256:512]
                    O11 = obt[:, 0:KO2, 0:SH]
                    O12 = obt[:, 0:KO2, SH:S]
                    O21 = obt[:, KO2:KO, 0:SH]
                    O22 = obt[:, KO2:KO, SH:S]

                    def occombo(a, bb, alu, name):
                        _ctr[0] += 1
                        t = ocp.tile([P, KO2, SH], BF16, tag=name, name=f"{name}{_ctr[0]}")
                        nc.vector.tensor_tensor(t[:], a, bb, alu)
                        return t

                    oc1 = occombo(O11, O22, ALU.add, "oc1")
                    oc3 = occombo(O12, O22, ALU.subtract, "oc3")
                    oc4 = occombo(O21, O11, ALU.subtract, "oc4")
                    oc6 = occombo(O11, O12, ALU.add, "oc6")
                    oc7 = occombo(O21, O22, ALU.add, "oc7")
                    # stationary operand per product (M1..M7)
                    ostat = [oc1[:], O11, oc3[:], oc4[:], O22, oc6[:], oc7[:]]

                    for gc in range(4):
                        ty11 = yacp.tile([P, 2, 512], F32, tag="y11", name=f"y11{b}{gc}")
                        ty12 = yacp.tile([P, 2, 512], F32, tag="y12", name=f"y12{b}{gc}")
                        ty21 = yacp.tile([P, 2, 512], F32, tag="y21", name=f"y21{b}{gc}")
                        ty22 = yacp.tile([P, 2, 512], F32, tag="y22", name=f"y22{b}{gc}")
                        for op in range(7):
                            u = punit()
                            for half in range(2):
                                slab = woslp.tile(
                                    [P, KO2 // 2, 512], BF16, tag="wosl",
                                    name=f"wo{b}{gc}{op}{half}",
                                )
                                nc.sync.dma_start(
                                    slab[:],
                                    wost_d[op, half * 8 : half * 8 + 8, :, gc * 512 : gc * 512 + 512]
                                    .rearrange("k p f -> p k f"),
                                )
                                for kk in range(KO2 // 2):
                                    ko = half * 8 + kk
                                    for tcc in range(2):
                                        nc.tensor.matmul(
                                            u[:, tcc, :],
                                            ostat[op][:, ko, tcc * P : (tcc + 1) * P],
                                            slab[:, kk, :],
                                            start=(ko == 0),
                                            stop=(ko == KO2 - 1),
                                        )
                            if op == 0:
                                nc.vector.tensor_copy(ty11[:], u[:])
                                nc.vector.tensor_copy(ty22[:], u[:])
                            elif op == 1:
                                nc.vector.tensor_copy(ty12[:], u[:])
                                nc.vector.tensor_tensor(ty22[:], ty22[:], u[:], ALU.subtract)
                            elif op == 2:
                                nc.vector.tensor_copy(ty21[:], u[:])
                                nc.vector.tensor_tensor(ty22[:], ty22[:], u[:], ALU.add)
                            elif op == 3:
                                nc.vector.tensor_tensor(ty11[:], ty11[:], u[:], ALU.add)
                                nc.vector.tensor_tensor(ty12[:], ty12[:], u[:], ALU.add)
                            elif op == 4:
                                nc.vector.tensor_tensor(ty11[:], ty11[:], u[:], ALU.subtract)
                                nc.vector.tensor_tensor(ty21[:], ty21[:], u[:], ALU.add)
                            elif op == 5:
                                nc.vector.tensor_tensor(ty22[:], ty22[:], u[:], ALU.add)
                            else:
                                nc.vector.tensor_tensor(ty11[:], ty11[:], u[:], ALU.add)
                        # y blocks: 11=[t0:256,g0half0] 12=[t0:256,half1]
                        #           21=[t256:512,half0] 22=[t256:512,half1]
                        g0a = gc * 512
                        g0b = EH + gc * 512
                        nc.sync.dma_start(
                            y_d[b, 0:SH, g0a : g0a + 512].rearrange("(j p) g -> p j g", p=P),
                            ty11[:],
                        )
                        nc.sync.dma_start(
                            y_d[b, 0:SH, g0b : g0b + 512].rearrange("(j p) g -> p j g", p=P),
                            ty12[:],
                        )
                        nc.sync.dma_start(
                            y_d[b, SH:S, g0a : g0a + 512].rearrange("(j p) g -> p j g", p=P),
                            ty21[:],
                        )
                        nc.sync.dma_start(
                            y_d[b, SH:S, g0b : g0b + 512].rearrange("(j p) g -> p j g", p=P),
                            ty22[:],
                        )

    nc.compile()
    return nc


def _prep_fast(x, Wq, bq, Wk, bk, Wv, bv, g_q, b_q, g_k, b_k, Wg, bg, Wo, bo):
    x = np.asarray(x, np.float32)

    def center(W):
        W4 = np.asarray(W, np.float32).reshape(H, HD, E)
        Wc = W4 - W4.mean(axis=1, keepdims=True)
        return Wc.reshape(E, E)

    def strassen_ops(G):
        """Standard A-side patterns of G [out, e]; stored [7, KO2, P(e), out-half]."""
        G11, G12 = G[:EH, :EH], G[:EH, EH:]
        G21, G22 = G[EH:, :EH], G[EH:, EH:]
        ops = [G11 + G22, G21 + G22, G11, G22, G11 + G12, G21 - G11, G12 - G22]
        out = np.empty((7, KO2, P, EH), BF)
        for i, op in enumerate(ops):
            out[i] = np.ascontiguousarray(op.T).reshape(KO2, P, EH).astype(BF)
        return out

    shared = {
        "wqst": strassen_ops(center(Wq)),
        "wkst": strassen_ops(center(Wk)),
        "wvst": strassen_ops(np.asarray(Wv, np.float32)),
        "wost": strassen_ops(np.asarray(Wo, np.float32)),
        "wgt": np.ascontiguousarray(
            np.asarray(Wg, np.float32).T.reshape(TC, P, 2 * HD)
        ).astype(BF),
    }

    in_maps = []
    for c in range(N_CORES):
        m = dict(shared)
        xsl = np.empty((NB, 7, KO2, P, SH), BF)
        for bi in range(NB):
            xT = np.ascontiguousarray(x[c * NB + bi].T)  # [E, S]
            B11, B12 = xT[:EH, :SH], xT[:EH, SH:]
            B21, B22 = xT[EH:, :SH], xT[EH:, SH:]
            ops = [B11 + B22, B11, B12 - B22, B21 - B11, B22, B11 + B12, B21 + B22]
            for i, op in enumerate(ops):
                xsl[bi, i] = np.ascontiguousarray(op).reshape(KO2, P, SH).astype(BF)
        m["xs"] = xsl
        in_maps.append(m)
    return in_maps


# =====================================================================
# general path: v1 direct implementation (biases / gains arbitrary)
# =====================================================================

def _bcast_ap(dram_ap, offset, n):
    return bass.AP(
        tensor=dram_ap.tensor, offset=dram_ap.offset + offset, ap=[[0, P], [1, n]]
    )


def _build_general_program():
    fast = False
    nc = bacc.Bacc("TRN2", target_bir_lowering=False, debug=False, num_devices=N_CORES)

    def dm(name, shape, dt, **kw):
        return nc.dram_tensor(name, shape, dt, **kw).ap()

    xt_d = dm("xt", [NB, KO, P, S], BF16, kind="ExternalInput")
    wqt_d = dm("wqt", [KO, P, E], BF16, kind="ExternalInput")
    wkt_d = dm("wkt", [KO, P, E], BF16, kind="ExternalInput")
    wvt_d = dm("wvt", [KO, P, E], BF16, kind="ExternalInput")
    wgt_d = dm("wgt", [TC, P, 2 * HD], BF16, kind="ExternalInput")
    wot_d = dm("wot", [KO, P, E], BF16, kind="ExternalInput")
    bqc_d = dm("bqc", [KO, P], F32, kind="ExternalInput")
    bkc_d = dm("bkc", [KO, P], F32, kind="ExternalInput")
    gq_d = dm("gq", [DC, P], F32, kind="ExternalInput")
    bqn_d = dm("bqn", [DC, P], F32, kind="ExternalInput")
    gk_d = dm("gk", [DC, P], F32, kind="ExternalInput")
    bkn_d = dm("bkn", [DC, P], F32, kind="ExternalInput")
    bgc_d = dm("bgc", [FC, P], F32, kind="ExternalInput")
    bv_d = dm("bv", [E], F32, kind="ExternalInput")
    bo_d = dm("bo", [E], F32, kind="ExternalInput")
    y_d = dm("y", [NB, S, E], F32, kind="ExternalOutput")
    rksc_d = dm("rksc", [NB * H, 512], F32)

    with tile.TileContext(nc) as tc:
        with (
            tc.tile_pool(name="singles", bufs=1) as singles,
            tc.tile_pool(name="xtp", bufs=1) as xtp,
            tc.tile_pool(name="obtp", bufs=1) as obtp,
            tc.tile_pool(name="wblk", bufs=6) as wblkp,
            tc.tile_pool(name="act", bufs=2) as actp,
            tc.tile_pool(name="act3", bufs=3) as actp3,
            tc.tile_pool(name="sqp", bufs=2) as sqp,
            tc.tile_pool(name="rows", bufs=6) as rowsp,
            tc.tile_pool(name="bc", bufs=3) as bcp,
            tc.tile_pool(name="bsl", bufs=2) as bslp,
            tc.tile_pool(name="cols", bufs=4) as colsp,
            tc.tile_pool(name="yout", bufs=2) as youtp,
            tc.tile_pool(name="ps", bufs=4, space="PSUM") as psp,
        ):
            nc.gpsimd.load_library(library_config.attn)

            ones_col = singles.tile([P, 1], BF16)
            nc.vector.memset(ones_col[:], 1.0)
            eps_qf = singles.tile([1, 1], F32)
            nc.vector.memset(eps_qf[:], float(HD * LN_EPS))
            eps_ln = singles.tile([1, 1], F32)
            nc.vector.memset(eps_ln[:], float(LN_EPS))
            eps_n2 = singles.tile([1, 1], F32)
            nc.vector.memset(eps_n2[:], float(NORM_EPS**2))
            wgt_sb = singles.tile([P, TC, 2 * HD], BF16)
            nc.sync.dma_start(wgt_sb[:], wgt_d.rearrange("t p f -> p t f"))

            def col_tile(dram, n):
                t = singles.tile([P, n], F32, name=f"ct_{dram.tensor.name}")
                nc.sync.dma_start(t[:], dram.rearrange("c p -> p c"))
                return t

            bqc_sb = col_tile(bqc_d, KO)
            bkc_sb = col_tile(bkc_d, KO)
            gq_sb = col_tile(gq_d, DC)
            bqn_sb = col_tile(bqn_d, DC)
            gk_sb = col_tile(gk_d, DC)
            bkn_sb = col_tile(bkn_d, DC)
            bgc_sb = col_tile(bgc_d, FC)

            _ctr = [0]

            def punit():
                _ctr[0] += 1
                return psp.tile([P, 2, 512], F32, tag="u", name=f"u{_ctr[0]}")

            def row(name):
                _ctr[0] += 1
                return rowsp.tile([1, 512], F32, tag="row", name=f"{name}{_ctr[0]}")

            def bcast128(row_ap, name):
                _ctr[0] += 1
                t = bcp.tile([P, 512], F32, tag="bc", name=f"{name}{_ctr[0]}")
                nc.gpsimd.partition_broadcast(t[:], row_ap)
                return t

            for b in range(NB):
                xt_sb = xtp.tile([P, KO, S], BF16, tag="xt")
                xt_splits = [(0, 1), (1, 4)] + [(4 * i, 4 * i + 4) for i in range(1, 8)]
                for lo, hi in xt_splits:
                    nc.gpsimd.dma_start(
                        xt_sb[:, lo:hi, :],
                        xt_d[b, lo:hi].rearrange("k p t -> p k t"),
                    )
                obt = obtp.tile([P, KO, S], BF16, tag="obt")

                for h in range(H):
                    f0 = h * HD

                    def wstream_blk(w_dram, kb, cols0, ncols):
                        _ctr[0] += 1
                        blk = wblkp.tile([P, 4, ncols], BF16, tag="wblk", name=f"w{_ctr[0]}")
                        nc.sync.dma_start(
                            blk[:],
                            w_dram[
                                4 * kb : 4 * kb + 4, :, cols0 : cols0 + ncols
                            ].rearrange("k p f -> p k f"),
                        )
                        return blk

                    def projT_mms(w_dram, units, kb):
                        blk = wstream_blk(w_dram, kb, f0, HD)
                        for j in range(4):
                            ko = 4 * kb + j
                            for dc in range(DC):
                                nc.tensor.matmul(
                                    units[dc // 2][:, dc % 2, :],
                                    blk[:, j, dc * P : (dc + 1) * P],
                                    xt_sb[:, ko, :],
                                    start=(ko == 0),
                                    stop=(ko == KO - 1),
                                )

                    def stats_mms(stat_slice, sq):
                        for dc in range(DC):
                            nc.tensor.matmul(
                                stat_slice,
                                ones_col[:],
                                sq[:, dc, :],
                                start=(dc == 0),
                                stop=(dc == DC - 1),
                            )

                    def consume_proj(units, bias_sb, name):
                        out_sb = actp.tile([P, DC, S], BF16, tag=name, name=f"{name}{h}{b}")
                        for dc in range(DC):
                            nc.vector.tensor_scalar(
                                out_sb[:, dc, :],
                                units[dc // 2][:, dc % 2, :],
                                bias_sb[:, h * DC + dc : h * DC + dc + 1],
                                None,
                                ALU.add,
                            )
                        sq = sqp.tile([P, DC, S], BF16, tag="sq", name=f"sq{name}{h}{b}")
                        nc.scalar.activation(sq[:], out_sb[:], AF.Square)
                        return out_sb, sq

                    qunits = [punit(), punit()]
                    for kb in range(4):
                        projT_mms(wqt_d, qunits, kb)
                    stat = psp.tile([1, 2, 512], F32, tag="u", name=f"st{h}{b}")
                    for kb in range(4, 8):
                        projT_mms(wqt_d, qunits, kb)
                    qc, sq_q = consume_proj(qunits, bqc_sb, "qc")

                    kunits = [punit(), punit()]
                    for kb in range(4):
                        projT_mms(wkt_d, kunits, kb)
                    stats_mms(stat[0:1, 0, :], sq_q)
                    for kb in range(4, 8):
                        projT_mms(wkt_d, kunits, kb)
                    kc, sq_k = consume_proj(kunits, bkc_sb, "kc")

                    sd_q = row("sdq")
                    nc.scalar.activation(
                        sd_q[:], stat[0:1, 0, :], AF.Sqrt,
                        bias=eps_ln[:], scale=float(1.0 / HD),
                    )
                    rq_row = row("rq")
                    nc.vector.reciprocal_approx_fast(rq_row[:], sd_q[:])
                    rqb = bcast128(rq_row[:], "rqb")

                    vunits = [punit(), punit()]
                    for kb in range(4):
                        blk = wstream_blk(wvt_d, kb, f0, HD)
                        for j in range(4):
                            ko = 4 * kb + j
                            for t_ in range(TC):
                                nc.tensor.matmul(
                                    vunits[t_ // 2][:, t_ % 2, :],
                                    xt_sb[:, ko, t_ * P : (t_ + 1) * P],
                                    blk[:, j, :],
                                    start=(ko == 0),
                                    stop=(ko == KO - 1),
                                )
                    stats_mms(stat[0:1, 1, :], sq_k)
                    for kb in range(4, 8):
                        blk = wstream_blk(wvt_d, kb, f0, HD)
                        for j in range(4):
                            ko = 4 * kb + j
                            for t_ in range(TC):
                                nc.tensor.matmul(
                                    vunits[t_ // 2][:, t_ % 2, :],
                                    xt_sb[:, ko, t_ * P : (t_ + 1) * P],
                                    blk[:, j, :],
                                    start=(ko == 0),
                                    stop=(ko == KO - 1),
                                )

                    sd_k = row("sdk")
                    nc.scalar.activation(
                        sd_k[:], stat[0:1, 1, :], AF.Sqrt,
                        bias=eps_ln[:], scale=float(1.0 / HD),
                    )
                    rk_row = row("rk")
                    nc.vector.reciprocal_approx_fast(rk_row[:], sd_k[:])
                    rkb = bcast128(rk_row[:], "rkb")
                    nc.vector.tensor_tensor(
                        kc[:], kc[:], rkb[:, None, :].to_broadcast((P, DC, S)), ALU.mult
                    )
                    for dc in range(DC):
                        nc.vector.tensor_scalar(
                            kc[:, dc, :],
                            kc[:, dc, :],
                            gk_sb[:, dc : dc + 1],
                            bkn_sb[:, dc : dc + 1],
                            ALU.mult,
                            ALU.add,
                        )
                    nc.vector.tensor_tensor(
                        qc[:], qc[:], rqb[:, None, :].to_broadcast((P, DC, S)), ALU.mult
                    )
                    for dc in range(DC):
                        nc.vector.tensor_scalar(
                            qc[:, dc, :],
                            qc[:, dc, :],
                            gq_sb[:, dc : dc + 1],
                            bqn_sb[:, dc : dc + 1],
                            ALU.mult,
                            ALU.add,
                        )

                    sunits = [punit(), punit()]
                    for t_ in range(TC):
                        for dc in range(DC):
                            nc.tensor.matmul(
                                sunits[t_ // 2][:, t_ % 2, :],
                                kc[:, dc, t_ * P : (t_ + 1) * P],
                                qc[:, dc, :],
                                start=(dc == 0),
                                stop=(dc == DC - 1),
                            )
                    sc = actp3.tile([P, TC, S], BF16, tag="sc", name=f"sc{h}{b}")
                    for u in range(2):
                        nc.vector.tensor_copy(sc[:, 2 * u : 2 * u + 2, :], sunits[u][:])

                    vc = actp.tile([P, TC, HD], BF16, tag="vc", name=f"vc{h}{b}")
                    bv_sl = bslp.tile([P, 512], F32, tag="bv", name=f"bv{h}{b}")
                    nc.sync.dma_start(bv_sl[:], _bcast_ap(bv_d, f0, 512))
                    for u in range(2):
                        nc.vector.tensor_tensor(
                            vc[:, 2 * u : 2 * u + 2, :],
                            vunits[u][:],
                            bv_sl[:, None, :].to_broadcast((P, 2, 512)),
                            ALU.add,
                        )

                    gunits = [punit(), punit()]
                    for i in range(DC):
                        fc = DC + i
                        for t_ in range(TC):
                            nc.tensor.matmul(
                                gunits[i // 2][:, i % 2, :],
                                wgt_sb[:, t_, fc * P : (fc + 1) * P],
                                sc[:, t_, :],
                                start=(t_ == 0),
                                stop=(t_ == TC - 1),
                            )
                    vunits2 = [punit(), punit()]
                    for i in range(DC):
                        for t_ in range(TC):
                            nc.tensor.matmul(
                                vunits2[i // 2][:, i % 2, :],
                                wgt_sb[:, t_, i * P : (i + 1) * P],
                                sc[:, t_, :],
                                start=(t_ == 0),
                                stop=(t_ == TC - 1),
                            )
                    gel = actp3.tile([P, DC, S], BF16, tag="gel", name=f"gel{h}{b}")
                    for i in range(DC):
                        nc.scalar.activation(
                            gel[:, i, :],
                            gunits[i // 2][:, i % 2, :],
                            AF.Gelu,
                            bias=bgc_sb[:, DC + i : DC + i + 1],
                        )
                    wv = actp3.tile([P, DC, S], BF16, tag="wv", name=f"wv{h}{b}")
                    for i in range(DC):
                        nc.vector.tensor_scalar(
                            wv[:, i, :],
                            vunits2[i // 2][:, i % 2, :],
                            bgc_sb[:, i : i + 1],
                            None,
                            ALU.add,
                        )
                    nc.vector.tensor_mul(wv[:], wv[:], gel[:])
                    sq_w = sqp.tile([P, DC, S], BF16, tag="sq", name=f"sqw{h}{b}")
                    nc.scalar.activation(sq_w[:], wv[:], AF.Square)

                    ounits = [punit(), punit()]
                    for t_ in range(TC):
                        for dc in range(DC):
                            nc.tensor.matmul(
                                ounits[dc // 2][:, dc % 2, :],
                                vc[:, t_, dc * P : (dc + 1) * P],
                                wv[:, t_, :],
                                start=(t_ == 0),
                                stop=(t_ == TC - 1),
                            )
                    stat2 = psp.tile([1, 2, 512], F32, tag="u", name=f"st2{h}{b}")
                    stats_mms(stat2[0:1, 0, :], sq_w)
                    nrow = row("nr")
                    nc.scalar.activation(
                        nrow[:], stat2[0:1, 0, :], AF.Sqrt, bias=eps_n2[:]
                    )
                    rr = row("rr")
                    nc.vector.reciprocal_approx_fast(rr[:], nrow[:])
                    rb = bcast128(rr[:], "rb")
                    for u in range(2):
                        nc.vector.tensor_tensor(
                            obt[:, h * DC + 2 * u : h * DC + 2 * u + 2, :],
                            ounits[u][:],
                            rb[:, None, :].to_broadcast((P, 2, 512)),
                            ALU.mult,
                        )

                for gb in range(NGB):
                    g0 = gb * 512
                    units = [punit(), punit()]
                    bo_sl = bslp.tile([P, 512], F32, tag="bo", name=f"bo{gb}{b}")
                    nc.sync.dma_start(bo_sl[:], _bcast_ap(bo_d, g0, 512))
                    for kb in range(8):
                        _ctr[0] += 1
                        blk = wblkp.tile([P, 4, 512], BF16, tag="wblk", name=f"wo{_ctr[0]}")
                        nc.sync.dma_start(
                            blk[:],
                            wot_d[4 * kb : 4 * kb + 4, :, g0 : g0 + 512].rearrange(
                                "k p f -> p k f"
                            ),
                        )
                        for j in range(4):
                            ko = 4 * kb + j
                            for t_ in range(TC):
                                nc.tensor.matmul(
                                    units[t_ // 2][:, t_ % 2, :],
                                    obt[:, ko, t_ * P : (t_ + 1) * P],
                                    blk[:, j, :],
                                    start=(ko == 0),
                                    stop=(ko == KO - 1),
                                )
                    for t_ in range(TC):
                        y_sb = youtp.tile([P, 512], F32, tag="y", name=f"y{gb}{t_}{b}")
                        nc.vector.tensor_add(
                            y_sb[:], units[t_ // 2][:, t_ % 2, :], bo_sl[:]
                        )
                        nc.sync.dma_start(
                            y_d[b, t_ * P : (t_ + 1) * P, g0 : g0 + 512], y_sb[:]
                        )

    nc.compile()
    return nc


def _prep_general(x, Wq, bq, Wk, bk, Wv, bv, g_q, b_q, g_k, b_k, Wg, bg, Wo, bo):
    x = np.asarray(x, np.float32)
    scale = 1.0 / np.sqrt(HD)

    def center(W, bvec):
        W4 = np.asarray(W, np.float32).reshape(H, HD, E)
        Wc = W4 - W4.mean(axis=1, keepdims=True)
        b4 = np.asarray(bvec, np.float32).reshape(H, HD)
        bc = b4 - b4.mean(axis=1, keepdims=True)
        return Wc.reshape(E, E), bc.reshape(E)

    Wq_c, bq_c = center(Wq, bq)
    Wk_c, bk_c = center(Wk, bk)

    def to_kpf(W):
        return np.ascontiguousarray(
            np.asarray(W, np.float32).T.reshape(KO, P, E)
        ).astype(BF)

    shared = {
        "wqt": to_kpf(Wq_c),
        "wkt": to_kpf(Wk_c),
        "wvt": to_kpf(np.asarray(Wv, np.float32)),
        "wot": to_kpf(np.asarray(Wo, np.float32)),
        "wgt": np.ascontiguousarray(
            np.asarray(Wg, np.float32).T.reshape(TC, P, 2 * HD)
        ).astype(BF),
        "bqc": bq_c.reshape(KO, P).astype(np.float32),
        "bkc": bk_c.reshape(KO, P).astype(np.float32),
        "gq": (np.asarray(g_q, np.float32) * scale).reshape(DC, P),
        "bqn": (np.asarray(b_q, np.float32) * scale).reshape(DC, P),
        "gk": np.asarray(g_k, np.float32).reshape(DC, P),
        "bkn": np.asarray(b_k, np.float32).reshape(DC, P),
        "bgc": np.asarray(bg, np.float32).reshape(FC, P),
        "bv": np.asarray(bv, np.float32),
        "bo": np.asarray(bo, np.float32),
    }
    shared = {k: np.ascontiguousarray(v) for k, v in shared.items()}

    xt = np.ascontiguousarray(x.transpose(0, 2, 1)).reshape(B, KO, P, S).astype(BF)
    in_maps = []
    for c in range(N_CORES):
        m = dict(shared)
        m["xt"] = np.ascontiguousarray(xt[c * NB : (c + 1) * NB])
        in_maps.append(m)
    return in_maps


# =====================================================================

_NC_CACHE = {}


def _get_nc(fast: bool):
    key = "fast" if fast else "general"
    if key not in _NC_CACHE:
        _install_ntff_hook()
        _NC_CACHE[key] = _build_fast_program() if fast else _build_general_program()
    return _NC_CACHE[key]


def _is_fast_case(bq, bk, bv, g_q, b_q, g_k, b_k, bg, bo):
    zeros = all(
        np.all(np.asarray(a) == 0.0) for a in (bq, bk, bv, b_q, b_k, bg, bo)
    )
    ones = all(np.all(np.asarray(a) == 1.0) for a in (g_q, g_k))
    return zeros and ones


def _run(trace, **inputs):
    fast = _is_fast_case(
        inputs["bq"], inputs["bk"], inputs["bv"], inputs["g_q"], inputs["b_q"],
        inputs["g_k"], inputs["b_k"], inputs["bg"], inputs["bo"],
    )
    nc = _get_nc(fast)
    in_maps = _prep_fast(**inputs) if fast else _prep_general(**inputs)
    res = run_bass_kernel_spmd(nc, in_maps, list(range(N_CORES)), trace=trace)
    out = np.empty((B, S, E), np.float32)
    for c in range(N_CORES):
        out[c * NB : (c + 1) * NB] = res.results[c]["y"]
    return out, res


def kernel(**inputs) -> np.ndarray:
    out, _ = _run(False, **inputs)
    return out


def kernel_profiled(**inputs):
    """Like kernel() but with NTFF tracing; returns (out, BassKernelResults)."""
    return _run(True, **inputs)


# revision 5
# speedup vs baseline: 1.1083x; 1.1083x over previous
"""Trainium2 Bass kernel for nn_MultiHeadAttention_833223655722.

Data-parallel over batch (16 / 8 cores = 2 per core). All matmuls bf16 with
fp32 PSUM accumulation. LayerNorm mean folded into centered projection
weights; rstd factors folded into the scores consume (fast path).

v2: one-level Strassen on all four E x E matmuls (7/8 multiply count):
  - Q/K projections (weight-stationary, qT[f,t] output): token split halves
    the moving free dim -> N=256 matmuls (still full rate, LDW hidden).
  - V projection and out-projection run the same scheme "executed
    transposed" (data-stationary, moving weight combos) so the weight
    output dim stays on the free axis -> N=512 matmuls.
  - The seven x-side operands (host-precomputed, bf16) serve Q/K as moving
    operands and V as stationary operands (identical storage layout).
  - W-side combos precomputed on host; C-block assembly on DVE from PSUM.
LN/L2 stats: ACT square + DVE pair-sum tree + a single ones-matmul
(1 matmul per stat instead of 4).

The general path (nonzero biases / non-unit gains) keeps the v1 direct
implementation.
"""

import sys
import types

import numpy as np
import ml_dtypes

import concourse.bass as bass
import concourse.mybir as mybir
import concourse.tile as tile
from concourse import bacc, library_config
from concourse import bass_utils
from concourse.bass_utils import run_bass_kernel_spmd

# ---------------------------------------------------------------- constants
B, S, E, H = 16, 512, 4096, 8
HD = E // H            # 512 (== S)
N_CORES = 8
NB = B // N_CORES      # 2 batches per core
P = 128
KO = E // P            # 32 contraction chunks over E
KO2 = KO // 2          # 16 chunks per e-half
TC = S // P            # 4 token chunks
DC = HD // P           # 4 head-dim chunks
FC = 2 * HD // P       # 8 GeGLU chunks
NGB = E // 512         # 8 out-proj column blocks
EH = E // 2            # 2048
SH = S // 2            # 256
LN_EPS = 1e-5
NORM_EPS = 1e-12

F32 = mybir.dt.float32
BF16 = mybir.dt.bfloat16
BF = ml_dtypes.bfloat16
AF = mybir.ActivationFunctionType
ALU = mybir.AluOpType


def _install_ntff_hook():
    """Register the NTFF profile hook missing from this image's antenv."""
    try:
        import antenv
        from trn_agent_boot.trn_boot import _ntff_profile_via_ctypes

        if "antenv.axon_hooks" in sys.modules:
            return
        hook = _ntff_profile_via_ctypes("/opt/axon/libaxon_pjrt.so")
        mod = types.ModuleType("antenv.axon_hooks")
        mod.get_axon_ntff_profile_hook = lambda: hook
        mod.set_axon_ntff_profile_hook = lambda h: None
        sys.modules["antenv.axon_hooks"] = mod
        antenv.axon_hooks = mod
        bass_utils.upload_artifacts = lambda tmpdir: tmpdir
    except Exception:
        pass


# =====================================================================
# fast path (all biases zero, gains one): Strassen build
# =====================================================================

def _build_fast_program():
    nc = bacc.Bacc("TRN2", target_bir_lowering=False, debug=False, num_devices=N_CORES)

    def dm(name, shape, dt, **kw):
        return nc.dram_tensor(name, shape, dt, **kw).ap()

    # seven x-side operands per batch, partition-major [op, P(e), ko(e), t]
    xs_d = dm("xs", [NB, 7, P, KO2, SH], BF16, kind="ExternalInput")
    # Q/K W-side operands, slab-major [op, mc, P(e), ko(e), f(128)]
    wqst_d = dm("wqst", [7, 16, P, KO2, P], BF16, kind="ExternalInput")
    wkst_d = dm("wkst", [7, 16, P, KO2, P], BF16, kind="ExternalInput")
    # V/O W-side operands, slab-major [op, gcol, P(e), ko(e), out(512)]
    wvst_d = dm("wvst", [7, 4, P, KO2, 512], BF16, kind="ExternalInput")
    wost_d = dm("wost", [7, 4, P, KO2, 512], BF16, kind="ExternalInput")
    wgt_d = dm("wgt", [TC, P, 2 * HD], BF16, kind="ExternalInput")
    y_d = dm("y", [NB, S, E], F32, kind="ExternalOutput")
    rksc_d = dm("rksc", [NB * H, 512], F32)

    with tile.TileContext(nc) as tc:
        with (
            tc.tile_pool(name="singles", bufs=1) as singles,
            tc.tile_pool(name="obtp", bufs=1) as obtp,
            tc.tile_pool(name="rows", bufs=4) as rowsp,
            tc.tile_pool(name="bc", bufs=3) as bcp,
            tc.tile_pool(name="cols", bufs=4) as colsp,
            tc.tile_pool(name="ps", bufs=4, space="PSUM") as psp,
        ):
            nc.gpsimd.load_library(library_config.attn)

            ones_col = singles.tile([P, 1], BF16)
            nc.vector.memset(ones_col[:], 1.0)
            eps_qf = singles.tile([1, 1], F32)
            nc.vector.memset(eps_qf[:], float(HD * LN_EPS))
            eps_ln = singles.tile([1, 1], F32)
            nc.vector.memset(eps_ln[:], float(LN_EPS))
            eps_n2 = singles.tile([1, 1], F32)
            nc.vector.memset(eps_n2[:], float(NORM_EPS**2))
            wgt_sb = singles.tile([P, TC, 2 * HD], BF16)
            nc.sync.dma_start(wgt_sb[:], wgt_d.rearrange("t p f -> p t f"))

            _ctr = [0]

            def punit():
                _ctr[0] += 1
                return psp.tile([P, 2, 512], F32, tag="u", name=f"u{_ctr[0]}")

            def row(name):
                _ctr[0] += 1
                return rowsp.tile([1, 512], F32, tag="row", name=f"{name}{_ctr[0]}")

            def bcast128(row_ap, name):
                _ctr[0] += 1
                t = bcp.tile([P, 512], F32, tag="bc", name=f"{name}{_ctr[0]}")
                nc.gpsimd.partition_broadcast(t[:], row_ap)
                return t

            for b in range(NB):
                obt = obtp.tile([P, KO, S], BF16, tag="obt", name=f"obt{b}")

                with (
                    tc.tile_pool(name=f"xsp{b}", bufs=1) as xsp,
                    tc.tile_pool(name=f"wsl{b}", bufs=3) as wslp,
                    tc.tile_pool(name=f"wvsl{b}", bufs=2) as wvslp,
                    tc.tile_pool(name=f"act{b}", bufs=2) as actp,
                    tc.tile_pool(name=f"act3{b}", bufs=2) as actp3,
                    tc.tile_pool(name=f"sqp{b}", bufs=1) as sqp,
                    tc.tile_pool(name=f"ctmp{b}", bufs=1) as ctmpp,
                    tc.tile_pool(name=f"vtmp{b}", bufs=1) as vtmpp,
                    tc.tile_pool(name=f"stt{b}", bufs=2) as sttp,
                ):
                    xs = xsp.tile([P, 7, KO2, SH], BF16, tag="xs", name=f"xs{b}")
                    for op in range(7):
                        nc.gpsimd.dma_start(xs[:, op], xs_d[b, op])

                    def wslab_qk(w_dram, op, mc):
                        _ctr[0] += 1
                        t = wslp.tile([P, KO2, P], BF16, tag="wsl", name=f"w{_ctr[0]}")
                        nc.sync.dma_start(t[:], w_dram[op, mc])
                        return t

                    def wslab_mov(w_dram, op, half, gcol):
                        """[P, 8, 512] slab: ko-half of a moving W operand."""
                        _ctr[0] += 1
                        t = wvslp.tile([P, KO2 // 2, 512], BF16, tag="wvsl", name=f"wv{_ctr[0]}")
                        nc.sync.dma_start(t[:], w_dram[op, gcol, :, half * 8 : half * 8 + 8, :])
                        return t

                    def stat_pre(src, name):
                        """src [P,4,512] bf16 -> ACT square + DVE tree -> [P,512] bf16."""
                        _ctr[0] += 1
                        sq = sqp.tile([P, DC, S], BF16, tag="sq", name=f"sq{name}{_ctr[0]}")
                        nc.scalar.activation(sq[:], src[:], AF.Square)
                        t0 = sttp.tile([P, 512], F32, tag="st0", name=f"st0{_ctr[0]}", bufs=1)
                        t1 = sttp.tile([P, 512], F32, tag="st1", name=f"st1{_ctr[0]}", bufs=1)
                        sbf = sttp.tile([P, 512], BF16, tag="stb", name=f"stb{_ctr[0]}")
                        nc.vector.tensor_tensor(t0[:], sq[:, 0, :], sq[:, 1, :], ALU.add)
                        nc.vector.tensor_tensor(t1[:], sq[:, 2, :], sq[:, 3, :], ALU.add)
                        nc.vector.tensor_tensor(sbf[:], t0[:], t1[:], ALU.add)
                        return sbf

                    def stat_mm(sbf, stat_slice):
                        nc.tensor.matmul(stat_slice, ones_col[:], sbf[:], start=True, stop=True)

                    # C-combo helper for Q/K quarters (one PSUM read per op)
                    def qk_combine(quarters, out_lo, out_hi, dc):
                        """quarters: list of 7 psum APs [P, 256] (M1..M7)."""
                        M = quarters
                        t = ctmpp.tile([P, SH], F32, tag="ct", name=f"ct{_ctr[0]}a")
                        _ctr[0] += 1
                        nc.vector.tensor_copy(t[:], M[0])
                        nc.vector.tensor_tensor(t[:], t[:], M[3], ALU.add)
                        nc.vector.tensor_tensor(t[:], t[:], M[4], ALU.subtract)
                        nc.vector.tensor_tensor(out_lo[:, dc, 0:SH], t[:], M[6], ALU.add)
                        t2 = ctmpp.tile([P, SH], F32, tag="ct2", name=f"ct{_ctr[0]}b")
                        _ctr[0] += 1
                        nc.vector.tensor_copy(t2[:], M[2])
                        nc.vector.tensor_tensor(out_lo[:, dc, SH:S], t2[:], M[4], ALU.add)
                        t3 = ctmpp.tile([P, SH], F32, tag="ct3", name=f"ct{_ctr[0]}c")
                        _ctr[0] += 1
                        nc.vector.tensor_copy(t3[:], M[1])
                        nc.vector.tensor_tensor(out_hi[:, dc, 0:SH], t3[:], M[3], ALU.add)
                        t4 = ctmpp.tile([P, SH], F32, tag="ct4", name=f"ct{_ctr[0]}d")
                        _ctr[0] += 1
                        nc.vector.tensor_copy(t4[:], M[0])
                        nc.vector.tensor_tensor(t4[:], t4[:], M[1], ALU.subtract)
                        nc.vector.tensor_tensor(t4[:], t4[:], M[2], ALU.add)
                        nc.vector.tensor_tensor(out_hi[:, dc, SH:S], t4[:], M[5], ALU.add)

                    def qk_proj(w_dram, g, out_lo, out_hi, name, inject=None):
                        """Strassen products for f-rows of heads (g, 4+g)."""
                        for mc in range(4 * g, 4 * g + 4):
                            if inject and (mc - 4 * g) in inject:
                                inject[mc - 4 * g]()
                            ua = punit()
                            ub = punit()
                            quarters = [
                                ua[:, 0, 0:SH], ua[:, 0, SH:512],
                                ua[:, 1, 0:SH], ua[:, 1, SH:512],
                                ub[:, 0, 0:SH], ub[:, 0, SH:512],
                                ub[:, 1, 0:SH],
                            ]
                            for op in range(7):
                                slab = wslab_qk(w_dram, op, mc)
                                for ko in range(KO2):
                                    nc.tensor.matmul(
                                        quarters[op],
                                        slab[:, ko, :],
                                        xs[:, op, ko, :],
                                        start=(ko == 0),
                                        stop=(ko == KO2 - 1),
                                    )
                            qk_combine(quarters, out_lo, out_hi, mc - 4 * g)

                    def v_proj(g, vc_lo, vc_hi, inject=None):
                        """Transposed-scheme products, d-columns of heads (g, 4+g)."""
                        t11 = vtmpp.tile([P, 2, 512], F32, tag="v11", name=f"v11{b}{g}")
                        t12 = vtmpp.tile([P, 2, 512], BF16, tag="v12", name=f"v12{b}{g}")
                        t21 = vtmpp.tile([P, 2, 512], BF16, tag="v21", name=f"v21{b}{g}")
                        t22 = vtmpp.tile([P, 2, 512], F32, tag="v22", name=f"v22{b}{g}")
                        for op in range(7):
                            if inject and op in inject:
                                inject[op]()
                            u = punit()
                            for half in range(2):
                                slab = wslab_mov(wvst_d, op, half, g)
                                for kk in range(KO2 // 2):
                                    ko = half * 8 + kk
                                    for tcc in range(2):
                                        nc.tensor.matmul(
                                            u[:, tcc, :],
                                            xs[:, op, ko, tcc * P : (tcc + 1) * P],
                                            slab[:, kk, :],
                                            start=(ko == 0),
                                            stop=(ko == KO2 - 1),
                                        )
                            # greedy C accumulation (P1..P7 = op 0..6)
                            if op == 0:
                                nc.vector.tensor_copy(t11[:], u[:])
                                nc.vector.tensor_copy(t22[:], u[:])
                            elif op == 1:
                                nc.vector.tensor_copy(t12[:], u[:])
                                nc.vector.tensor_tensor(t22[:], t22[:], u[:], ALU.subtract)
                            elif op == 2:
                                nc.vector.tensor_copy(t21[:], u[:])
                                nc.vector.tensor_tensor(t22[:], t22[:], u[:], ALU.add)
                            elif op == 3:
                                nc.vector.tensor_tensor(t11[:], t11[:], u[:], ALU.add)
                                nc.vector.tensor_tensor(vc_hi[:, 0:2, :], t12[:], u[:], ALU.add)
                            elif op == 4:
                                nc.vector.tensor_tensor(t11[:], t11[:], u[:], ALU.subtract)
                                nc.vector.tensor_tensor(vc_lo[:, 2:4, :], t21[:], u[:], ALU.add)
                            elif op == 5:
                                nc.vector.tensor_tensor(vc_hi[:, 2:4, :], t22[:], u[:], ALU.add)
                            else:
                                nc.vector.tensor_tensor(vc_lo[:, 0:2, :], t11[:], u[:], ALU.add)

                    # ---------------- attention for one head ----------------
                    def attention(h, qc, kc, vc, rqb, rk_cols):
                        sunits = [punit(), punit()]
                        for t_ in range(TC):
                            for dc in range(DC):
                                nc.tensor.matmul(
                                    sunits[t_ // 2][:, t_ % 2, :],
                                    kc[:, dc, t_ * P : (t_ + 1) * P],
                                    qc[:, dc, :],
                                    start=(dc == 0),
                                    stop=(dc == DC - 1),
                                )
                        sc = actp3.tile([P, TC, S], BF16, tag="sc", name=f"sc{h}{b}")
                        for t_ in range(TC):
                            nc.vector.scalar_tensor_tensor(
                                sc[:, t_, :],
                                sunits[t_ // 2][:, t_ % 2, :],
                                rk_cols[:, t_ : t_ + 1],
                                rqb[:],
                                ALU.mult,
                                ALU.mult,
                            )
                        gunits = [punit(), punit()]
                        for i in range(DC):
                            fc = DC + i
                            for t_ in range(TC):
                                nc.tensor.matmul(
                                    gunits[i // 2][:, i % 2, :],
                                    wgt_sb[:, t_, fc * P : (fc + 1) * P],
                                    sc[:, t_, :],
                                    start=(t_ == 0),
                                    stop=(t_ == TC - 1),
                                )
                        vunits2 = [punit(), punit()]
                        for i in range(DC):
                            for t_ in range(TC):
                                nc.tensor.matmul(
                                    vunits2[i // 2][:, i % 2, :],
                                    wgt_sb[:, t_, i * P : (i + 1) * P],
                                    sc[:, t_, :],
                                    start=(t_ == 0),
                                    stop=(t_ == TC - 1),
                                )
                        gel = actp3.tile([P, DC, S], BF16, tag="gel", name=f"gel{h}{b}", bufs=1)
                        for i in range(DC):
                            nc.scalar.activation(
                                gel[:, i, :],
                                gunits[i // 2][:, i % 2, :],
                                AF.Gelu,
                                bias=0.0,
                            )
                        wv = actp3.tile([P, DC, S], BF16, tag="wv", name=f"wv{h}{b}", bufs=1)
                        for u in range(2):
                            nc.vector.tensor_copy(
                                wv[:, 2 * u : 2 * u + 2, :], vunits2[u][:]
                            )
                            nc.vector.tensor_mul(
                                wv[:, 2 * u : 2 * u + 2, :],
                                wv[:, 2 * u : 2 * u + 2, :],
                                gel[:, 2 * u : 2 * u + 2, :],
                            )
                        # L2 stats: tree now, matmul after the out MMs
                        sbf_w = stat_pre(wv, f"w{h}")
                        stat2 = psp.tile([1, 2, 512], F32, tag="u", name=f"st2{h}{b}")
                        # out matmuls (t-major)
                        ounits = [punit(), punit()]
                        for t_ in range(TC):
                            for dc in range(DC):
                                nc.tensor.matmul(
                                    ounits[dc // 2][:, dc % 2, :],
                                    vc[:, t_, dc * P : (dc + 1) * P],
                                    wv[:, t_, :],
                                    start=(t_ == 0),
                                    stop=(t_ == TC - 1),
                                )
                        stat_mm(sbf_w, stat2[0:1, 0, :])
                        nrow = row("nr")
                        nc.scalar.activation(
                            nrow[:], stat2[0:1, 0, :], AF.Sqrt, bias=eps_n2[:]
                        )
                        rr = row("rr")
                        nc.vector.reciprocal_approx_fast(rr[:], nrow[:])
                        rb = bcast128(rr[:], "rb")
                        for u in range(2):
                            nc.vector.tensor_tensor(
                                obt[:, h * DC + 2 * u : h * DC + 2 * u + 2, :],
                                ounits[u][:],
                                rb[:, None, :].to_broadcast((P, 2, 512)),
                                ALU.mult,
                            )

                    # =============== head-pair groups ===============
                    for g in range(4):
                        hl, hh = g, 4 + g

                        qc_lo = actp.tile([P, DC, S], BF16, tag="qc", name=f"qc{hl}{b}")
                        qc_hi = actp.tile([P, DC, S], BF16, tag="qc", name=f"qc{hh}{b}")
                        qk_proj(wqst_d, g, qc_lo, qc_hi, "q")

                        kc_lo = actp.tile([P, DC, S], BF16, tag="kc", name=f"kc{hl}{b}")
                        kc_hi = actp.tile([P, DC, S], BF16, tag="kc", name=f"kc{hh}{b}")
                        # q stats + rows interleaved into the K product stream
                        sbf_ql = stat_pre(qc_lo, f"ql{g}")
                        sbf_qh = stat_pre(qc_hi, f"qh{g}")
                        stat = psp.tile([1, 2, 512], F32, tag="u", name=f"stq{g}{b}")
                        rqbs = []

                        def q_stats_mm():
                            stat_mm(sbf_ql, stat[0:1, 0, :])
                            stat_mm(sbf_qh, stat[0:1, 1, :])

                        def q_rows():
                            for slot in range(2):
                                sd_q = row("sdq")
                                nc.scalar.activation(
                                    sd_q[:], stat[0:1, slot, :], AF.Sqrt, bias=eps_qf[:]
                                )
                                rq_row = row("rq")
                                nc.vector.reciprocal_approx_fast(rq_row[:], sd_q[:])
                                rqbs.append(bcast128(rq_row[:], "rqb"))

                        qk_proj(wkst_d, g, kc_lo, kc_hi, "k",
                                inject={1: q_stats_mm, 2: q_rows})

                        vc_lo = actp.tile([P, TC, HD], BF16, tag="vc", name=f"vc{hl}{b}")
                        vc_hi = actp.tile([P, TC, HD], BF16, tag="vc", name=f"vc{hh}{b}")
                        # k stats + rows interleaved into the V product stream
                        sbf_kl = stat_pre(kc_lo, f"kl{g}")
                        sbf_kh = stat_pre(kc_hi, f"kh{g}")
                        statk = psp.tile([1, 2, 512], F32, tag="u", name=f"stk{g}{b}")
                        rkcs = []

                        def k_stats_mm():
                            stat_mm(sbf_kl, statk[0:1, 0, :])
                            stat_mm(sbf_kh, statk[0:1, 1, :])

                        def k_rows():
                            for slot, h in ((0, hl), (1, hh)):
                                sd_k = row("sdk")
                                nc.scalar.activation(
                                    sd_k[:], statk[0:1, slot, :], AF.Sqrt,
                                    bias=eps_ln[:], scale=float(1.0 / HD),
                                )
                                idx = b * H + h
                                nc.sync.dma_start(rksc_d[idx : idx + 1, :], sd_k[:])
                                sd_cols = colsp.tile([P, TC], F32, tag="cols", name=f"sdc{h}{b}")
                                nc.sync.dma_start(
                                    sd_cols[:], rksc_d[idx].rearrange("(c p) -> p c", p=P)
                                )
                                rk_cols = colsp.tile([P, TC], F32, tag="cols", name=f"rkc{h}{b}")
                                nc.vector.reciprocal_approx_fast(rk_cols[:], sd_cols[:])
                                rkcs.append(rk_cols)

                        v_proj(g, vc_lo, vc_hi,
                               inject={2: k_stats_mm, 4: k_rows})

                        attention(hl, qc_lo, kc_lo, vc_lo, rqbs[0], rkcs[0])
                        attention(hh, qc_hi, kc_hi, vc_hi, rqbs[1], rkcs[1])

                # ---------------- out-projection (Strassen) ----------------
                with (
                    tc.tile_pool(name=f"ocp{b}", bufs=1) as ocp,
                    tc.tile_pool(name=f"wosl{b}", bufs=3) as woslp,
                    tc.tile_pool(name=f"yac{b}", bufs=1) as yacp,
                ):
                    # obtT blocks: O11=obt[:,0:16,0:256] O12=[...,256:512]
                    #              O21=obt[:,16:32,0:256] O22=[...,16:32,256:512]
                    O11 = obt[:, 0:KO2, 0:SH]
                    O12 = obt[:, 0:KO2, SH:S]
                    O21 = obt[:, KO2:KO, 0:SH]
                    O22 = obt[:, KO2:KO, SH:S]

                    def occombo(a, bb, alu, name):
                        _ctr[0] += 1
                        t = ocp.tile([P, KO2, SH], BF16, tag=name, name=f"{name}{_ctr[0]}")
                        nc.vector.tensor_tensor(t[:], a, bb, alu)
                        return t

                    oc1 = occombo(O11, O22, ALU.add, "oc1")
                    oc3 = occombo(O12, O22, ALU.subtract, "oc3")
                    oc4 = occombo(O21, O11, ALU.subtract, "oc4")
                    oc6 = occombo(O11, O12, ALU.add, "oc6")
                    oc7 = occombo(O21, O22, ALU.add, "oc7")
                    # stationary operand per product (M1..M7)
                    ostat = [oc1[:], O11, oc3[:], oc4[:], O22, oc6[:], oc7[:]]

                    for gc in range(4):
                        ty11 = yacp.tile([P, 2, 512], F32, tag="y11", name=f"y11{b}{gc}")
                        ty12 = yacp.tile([P, 2, 512], F32, tag="y12", name=f"y12{b}{gc}")
                        ty21 = yacp.tile([P, 2, 512], F32, tag="y21", name=f"y21{b}{gc}")
                        ty22 = yacp.tile([P, 2, 512], F32, tag="y22", name=f"y22{b}{gc}")
                        for op in range(7):
                            u = punit()
                            for half in range(2):
                                slab = woslp.tile(
                                    [P, KO2 // 2, 512], BF16, tag="wosl",
                                    name=f"wo{b}{gc}{op}{half}",
                                )
                                nc.sync.dma_start(
                                    slab[:],
                                    wost_d[op, gc, :, half * 8 : half * 8 + 8, :],
                                )
                                for kk in range(KO2 // 2):
                                    ko = half * 8 + kk
                                    for tcc in range(2):
                                        nc.tensor.matmul(
                                            u[:, tcc, :],
                                            ostat[op][:, ko, tcc * P : (tcc + 1) * P],
                                            slab[:, kk, :],
                                            start=(ko == 0),
                                            stop=(ko == KO2 - 1),
                                        )
                            if op == 0:
                                nc.vector.tensor_copy(ty11[:], u[:])
                                nc.vector.tensor_copy(ty22[:], u[:])
                            elif op == 1:
                                nc.vector.tensor_copy(ty12[:], u[:])
                                nc.vector.tensor_tensor(ty22[:], ty22[:], u[:], ALU.subtract)
                            elif op == 2:
                                nc.vector.tensor_copy(ty21[:], u[:])
                                nc.vector.tensor_tensor(ty22[:], ty22[:], u[:], ALU.add)
                            elif op == 3:
                                nc.vector.tensor_tensor(ty11[:], ty11[:], u[:], ALU.add)
                                nc.vector.tensor_tensor(ty12[:], ty12[:], u[:], ALU.add)
                            elif op == 4:
                                nc.vector.tensor_tensor(ty11[:], ty11[:], u[:], ALU.subtract)
                                nc.vector.tensor_tensor(ty21[:], ty21[:], u[:], ALU.add)
                            elif op == 5:
                                nc.vector.tensor_tensor(ty22[:], ty22[:], u[:], ALU.add)
                            else:
                                nc.vector.tensor_tensor(ty11[:], ty11[:], u[:], ALU.add)
                        # y blocks: 11=[t0:256,g0half0] 12=[t0:256,half1]
                        #           21=[t256:512,half0] 22=[t256:512,half1]
                        g0a = gc * 512
                        g0b = EH + gc * 512
                        nc.sync.dma_start(
                            y_d[b, 0:SH, g0a : g0a + 512].rearrange("(j p) g -> p j g", p=P),
                            ty11[:],
                        )
                        nc.sync.dma_start(
                            y_d[b, 0:SH, g0b : g0b + 512].rearrange("(j p) g -> p j g", p=P),
                            ty12[:],
                        )
                        nc.sync.dma_start(
                            y_d[b, SH:S, g0a : g0a + 512].rearrange("(j p) g -> p j g", p=P),
                            ty21[:],
                        )
                        nc.sync.dma_start(
                            y_d[b, SH:S, g0b : g0b + 512].rearrange("(j p) g -> p j g", p=P),
                            ty22[:],
                        )

    nc.compile()
    return nc


def _prep_fast(x, Wq, bq, Wk, bk, Wv, bv, g_q, b_q, g_k, b_k, Wg, bg, Wo, bo):
    x = np.asarray(x, np.float32)

    def center(W):
        W4 = np.asarray(W, np.float32).reshape(H, HD, E)
        Wc = W4 - W4.mean(axis=1, keepdims=True)
        return Wc.reshape(E, E)

    def strassen_ops(G, blk):
        """Standard A-side patterns of G [out, e]; slab-major
        [7, EH//blk, P(e), KO2, blk] so each slab is contiguous per partition."""
        G11, G12 = G[:EH, :EH], G[:EH, EH:]
        G21, G22 = G[EH:, :EH], G[EH:, EH:]
        ops = [G11 + G22, G21 + G22, G11, G22, G11 + G12, G21 - G11, G12 - G22]
        nblk = EH // blk
        out = np.empty((7, nblk, P, KO2, blk), BF)
        for i, op in enumerate(ops):
            # op.T: [e 2048, out 2048] -> [KO2, P, nblk, blk] -> (nblk, P, KO2, blk)
            t = np.ascontiguousarray(op.T).reshape(KO2, P, nblk, blk)
            out[i] = t.transpose(2, 1, 0, 3).astype(BF)
        return out

    shared = {
        "wqst": strassen_ops(center(Wq), P),
        "wkst": strassen_ops(center(Wk), P),
        "wvst": strassen_ops(np.asarray(Wv, np.float32), 512),
        "wost": strassen_ops(np.asarray(Wo, np.float32), 512),
        "wgt": np.ascontiguousarray(
            np.asarray(Wg, np.float32).T.reshape(TC, P, 2 * HD)
        ).astype(BF),
    }

    in_maps = []
    for c in range(N_CORES):
        m = dict(shared)
        xsl = np.empty((NB, 7, P, KO2, SH), BF)
        for bi in range(NB):
            xT = np.ascontiguousarray(x[c * NB + bi].T)  # [E, S]
            B11, B12 = xT[:EH, :SH], xT[:EH, SH:]
            B21, B22 = xT[EH:, :SH], xT[EH:, SH:]
            ops = [B11 + B22, B11, B12 - B22, B21 - B11, B22, B11 + B12, B21 + B22]
            for i, op in enumerate(ops):
                # [2048e, SH] -> [KO2, P, SH] -> [P, KO2, SH]
                xsl[bi, i] = (
                    np.ascontiguousarray(op).reshape(KO2, P, SH)
                    .transpose(1, 0, 2).astype(BF)
                )
        m["xs"] = xsl
        in_maps.append(m)
    return in_maps


# =====================================================================
# general path: v1 direct implementation (biases / gains arbitrary)
# =====================================================================

def _bcast_ap(dram_ap, offset, n):
    return bass.AP(
        tensor=dram_ap.tensor, offset=dram_ap.offset + offset, ap=[[0, P], [1, n]]
    )


def _build_general_program():
    fast = False
    nc = bacc.Bacc("TRN2", target_bir_lowering=False, debug=False, num_devices=N_CORES)

    def dm(name, shape, dt, **kw):
        return nc.dram_tensor(name, shape, dt, **kw).ap()

    xt_d = dm("xt", [NB, KO, P, S], BF16, kind="ExternalInput")
    wqt_d = dm("wqt", [KO, P, E], BF16, kind="ExternalInput")
    wkt_d = dm("wkt", [KO, P, E], BF16, kind="ExternalInput")
    wvt_d = dm("wvt", [KO, P, E], BF16, kind="ExternalInput")
    wgt_d = dm("wgt", [TC, P, 2 * HD], BF16, kind="ExternalInput")
    wot_d = dm("wot", [KO, P, E], BF16, kind="ExternalInput")
    bqc_d = dm("bqc", [KO, P], F32, kind="ExternalInput")
    bkc_d = dm("bkc", [KO, P], F32, kind="ExternalInput")
    gq_d = dm("gq", [DC, P], F32, kind="ExternalInput")
    bqn_d = dm("bqn", [DC, P], F32, kind="ExternalInput")
    gk_d = dm("gk", [DC, P], F32, kind="ExternalInput")
    bkn_d = dm("bkn", [DC, P], F32, kind="ExternalInput")
    bgc_d = dm("bgc", [FC, P], F32, kind="ExternalInput")
    bv_d = dm("bv", [E], F32, kind="ExternalInput")
    bo_d = dm("bo", [E], F32, kind="ExternalInput")
    y_d = dm("y", [NB, S, E], F32, kind="ExternalOutput")
    rksc_d = dm("rksc", [NB * H, 512], F32)

    with tile.TileContext(nc) as tc:
        with (
            tc.tile_pool(name="singles", bufs=1) as singles,
            tc.tile_pool(name="xtp", bufs=1) as xtp,
            tc.tile_pool(name="obtp", bufs=1) as obtp,
            tc.tile_pool(name="wblk", bufs=6) as wblkp,
            tc.tile_pool(name="act", bufs=2) as actp,
            tc.tile_pool(name="act3", bufs=3) as actp3,
            tc.tile_pool(name="sqp", bufs=2) as sqp,
            tc.tile_pool(name="rows", bufs=6) as rowsp,
            tc.tile_pool(name="bc", bufs=3) as bcp,
            tc.tile_pool(name="bsl", bufs=2) as bslp,
            tc.tile_pool(name="cols", bufs=4) as colsp,
            tc.tile_pool(name="yout", bufs=2) as youtp,
            tc.tile_pool(name="ps", bufs=4, space="PSUM") as psp,
        ):
            nc.gpsimd.load_library(library_config.attn)

            ones_col = singles.tile([P, 1], BF16)
            nc.vector.memset(ones_col[:], 1.0)
            eps_qf = singles.tile([1, 1], F32)
            nc.vector.memset(eps_qf[:], float(HD * LN_EPS))
            eps_ln = singles.tile([1, 1], F32)
            nc.vector.memset(eps_ln[:], float(LN_EPS))
            eps_n2 = singles.tile([1, 1], F32)
            nc.vector.memset(eps_n2[:], float(NORM_EPS**2))
            wgt_sb = singles.tile([P, TC, 2 * HD], BF16)
            nc.sync.dma_start(wgt_sb[:], wgt_d.rearrange("t p f -> p t f"))

            def col_tile(dram, n):
                t = singles.tile([P, n], F32, name=f"ct_{dram.tensor.name}")
                nc.sync.dma_start(t[:], dram.rearrange("c p -> p c"))
                return t

            bqc_sb = col_tile(bqc_d, KO)
            bkc_sb = col_tile(bkc_d, KO)
            gq_sb = col_tile(gq_d, DC)
            bqn_sb = col_tile(bqn_d, DC)
            gk_sb = col_tile(gk_d, DC)
            bkn_sb = col_tile(bkn_d, DC)
            bgc_sb = col_tile(bgc_d, FC)

            _ctr = [0]

            def punit():
                _ctr[0] += 1
                return psp.tile([P, 2, 512], F32, tag="u", name=f"u{_ctr[0]}")

            def row(name):
                _ctr[0] += 1
                return rowsp.tile([1, 512], F32, tag="row", name=f"{name}{_ctr[0]}")

            def bcast128(row_ap, name):
                _ctr[0] += 1
                t = bcp.tile([P, 512], F32, tag="bc", name=f"{name}{_ctr[0]}")
                nc.gpsimd.partition_broadcast(t[:], row_ap)
                return t

            for b in range(NB):
                xt_sb = xtp.tile([P, KO, S], BF16, tag="xt")
                xt_splits = [(0, 1), (1, 4)] + [(4 * i, 4 * i + 4) for i in range(1, 8)]
                for lo, hi in xt_splits:
                    nc.gpsimd.dma_start(
                        xt_sb[:, lo:hi, :],
                        xt_d[b, lo:hi].rearrange("k p t -> p k t"),
                    )
                obt = obtp.tile([P, KO, S], BF16, tag="obt")

                for h in range(H):
                    f0 = h * HD

                    def wstream_blk(w_dram, kb, cols0, ncols):
                        _ctr[0] += 1
                        blk = wblkp.tile([P, 4, ncols], BF16, tag="wblk", name=f"w{_ctr[0]}")
                        nc.sync.dma_start(
                            blk[:],
                            w_dram[
                                4 * kb : 4 * kb + 4, :, cols0 : cols0 + ncols
                            ].rearrange("k p f -> p k f"),
                        )
                        return blk

                    def projT_mms(w_dram, units, kb):
                        blk = wstream_blk(w_dram, kb, f0, HD)
                        for j in range(4):
                            ko = 4 * kb + j
                            for dc in range(DC):
                                nc.tensor.matmul(
                                    units[dc // 2][:, dc % 2, :],
                                    blk[:, j, dc * P : (dc + 1) * P],
                                    xt_sb[:, ko, :],
                                    start=(ko == 0),
                                    stop=(ko == KO - 1),
                                )

                    def stats_mms(stat_slice, sq):
                        for dc in range(DC):
                            nc.tensor.matmul(
                                stat_slice,
                                ones_col[:],
                                sq[:, dc, :],
                                start=(dc == 0),
                                stop=(dc == DC - 1),
                            )

                    def consume_proj(units, bias_sb, name):
                        out_sb = actp.tile([P, DC, S], BF16, tag=name, name=f"{name}{h}{b}")
                        for dc in range(DC):
                            nc.vector.tensor_scalar(
                                out_sb[:, dc, :],
                                units[dc // 2][:, dc % 2, :],
                                bias_sb[:, h * DC + dc : h * DC + dc + 1],
                                None,
                                ALU.add,
                            )
                        sq = sqp.tile([P, DC, S], BF16, tag="sq", name=f"sq{name}{h}{b}")
                        nc.scalar.activation(sq[:], out_sb[:], AF.Square)
                        return out_sb, sq

                    qunits = [punit(), punit()]
                    for kb in range(4):
                        projT_mms(wqt_d, qunits, kb)
                    stat = psp.tile([1, 2, 512], F32, tag="u", name=f"st{h}{b}")
                    for kb in range(4, 8):
                        projT_mms(wqt_d, qunits, kb)
                    qc, sq_q = consume_proj(qunits, bqc_sb, "qc")

                    kunits = [punit(), punit()]
                    for kb in range(4):
                        projT_mms(wkt_d, kunits, kb)
                    stats_mms(stat[0:1, 0, :], sq_q)
                    for kb in range(4, 8):
                        projT_mms(wkt_d, kunits, kb)
                    kc, sq_k = consume_proj(kunits, bkc_sb, "kc")

                    sd_q = row("sdq")
                    nc.scalar.activation(
                        sd_q[:], stat[0:1, 0, :], AF.Sqrt,
                        bias=eps_ln[:], scale=float(1.0 / HD),
                    )
                    rq_row = row("rq")
                    nc.vector.reciprocal_approx_fast(rq_row[:], sd_q[:])
                    rqb = bcast128(rq_row[:], "rqb")

                    vunits = [punit(), punit()]
                    for kb in range(4):
                        blk = wstream_blk(wvt_d, kb, f0, HD)
                        for j in range(4):
                            ko = 4 * kb + j
                            for t_ in range(TC):
                                nc.tensor.matmul(
                                    vunits[t_ // 2][:, t_ % 2, :],
                                    xt_sb[:, ko, t_ * P : (t_ + 1) * P],
                                    blk[:, j, :],
                                    start=(ko == 0),
                                    stop=(ko == KO - 1),
                                )
                    stats_mms(stat[0:1, 1, :], sq_k)
                    for kb in range(4, 8):
                        blk = wstream_blk(wvt_d, kb, f0, HD)
                        for j in range(4):
                            ko = 4 * kb + j
                            for t_ in range(TC):
                                nc.tensor.matmul(
                                    vunits[t_ // 2][:, t_ % 2, :],
                                    xt_sb[:, ko, t_ * P : (t_ + 1) * P],
                                    blk[:, j, :],
                                    start=(ko == 0),
                                    stop=(ko == KO - 1),
                                )

                    sd_k = row("sdk")
                    nc.scalar.activation(
                        sd_k[:], stat[0:1, 1, :], AF.Sqrt,
                        bias=eps_ln[:], scale=float(1.0 / HD),
                    )
                    rk_row = row("rk")
                    nc.vector.reciprocal_approx_fast(rk_row[:], sd_k[:])
                    rkb = bcast128(rk_row[:], "rkb")
                    nc.vector.tensor_tensor(
                        kc[:], kc[:], rkb[:, None, :].to_broadcast((P, DC, S)), ALU.mult
                    )
                    for dc in range(DC):
                        nc.vector.tensor_scalar(
                            kc[:, dc, :],
                            kc[:, dc, :],
                            gk_sb[:, dc : dc + 1],
                            bkn_sb[:, dc : dc + 1],
                            ALU.mult,
                            ALU.add,
                        )
                    nc.vector.tensor_tensor(
                        qc[:], qc[:], rqb[:, None, :].to_broadcast((P, DC, S)), ALU.mult
                    )
                    for dc in range(DC):
                        nc.vector.tensor_scalar(
                            qc[:, dc, :],
                            qc[:, dc, :],
                            gq_sb[:, dc : dc + 1],
                            bqn_sb[:, dc : dc + 1],
                            ALU.mult,
                            ALU.add,
                        )

                    sunits = [punit(), punit()]
                    for t_ in range(TC):
                        for dc in range(DC):
                            nc.tensor.matmul(
                                sunits[t_ // 2][:, t_ % 2, :],
                                kc[:, dc, t_ * P : (t_ + 1) * P],
                                qc[:, dc, :],
                                start=(dc == 0),
                                stop=(dc == DC - 1),
                            )
                    sc = actp3.tile([P, TC, S], BF16, tag="sc", name=f"sc{h}{b}")
                    for u in range(2):
                        nc.vector.tensor_copy(sc[:, 2 * u : 2 * u + 2, :], sunits[u][:])

                    vc = actp.tile([P, TC, HD], BF16, tag="vc", name=f"vc{h}{b}")
                    bv_sl = bslp.tile([P, 512], F32, tag="bv", name=f"bv{h}{b}")
                    nc.sync.dma_start(bv_sl[:], _bcast_ap(bv_d, f0, 512))
                    for u in range(2):
                        nc.vector.tensor_tensor(
                            vc[:, 2 * u : 2 * u + 2, :],
                            vunits[u][:],
                            bv_sl[:, None, :].to_broadcast((P, 2, 512)),
                            ALU.add,
                        )

                    gunits = [punit(), punit()]
                    for i in range(DC):
                        fc = DC + i
                        for t_ in range(TC):
                            nc.tensor.matmul(
                                gunits[i // 2][:, i % 2, :],
                                wgt_sb[:, t_, fc * P : (fc + 1) * P],
                                sc[:, t_, :],
                                start=(t_ == 0),
                                stop=(t_ == TC - 1),
                            )
                    vunits2 = [punit(), punit()]
                    for i in range(DC):
                        for t_ in range(TC):
                            nc.tensor.matmul(
                                vunits2[i // 2][:, i % 2, :],
                                wgt_sb[:, t_, i * P : (i + 1) * P],
                                sc[:, t_, :],
                                start=(t_ == 0),
                                stop=(t_ == TC - 1),
                            )
                    gel = actp3.tile([P, DC, S], BF16, tag="gel", name=f"gel{h}{b}")
                    for i in range(DC):
                        nc.scalar.activation(
                            gel[:, i, :],
                            gunits[i // 2][:, i % 2, :],
                            AF.Gelu,
                            bias=bgc_sb[:, DC + i : DC + i + 1],
                        )
                    wv = actp3.tile([P, DC, S], BF16, tag="wv", name=f"wv{h}{b}")
                    for i in range(DC):
                        nc.vector.tensor_scalar(
                            wv[:, i, :],
                            vunits2[i // 2][:, i % 2, :],
                            bgc_sb[:, i : i + 1],
                            None,
                            ALU.add,
                        )
                    nc.vector.tensor_mul(wv[:], wv[:], gel[:])
                    sq_w = sqp.tile([P, DC, S], BF16, tag="sq", name=f"sqw{h}{b}")
                    nc.scalar.activation(sq_w[:], wv[:], AF.Square)

                    ounits = [punit(), punit()]
                    for t_ in range(TC):
                        for dc in range(DC):
                            nc.tensor.matmul(
                                ounits[dc // 2][:, dc % 2, :],
                                vc[:, t_, dc * P : (dc + 1) * P],
                                wv[:, t_, :],
                                start=(t_ == 0),
                                stop=(t_ == TC - 1),
                            )
                    stat2 = psp.tile([1, 2, 512], F32, tag="u", name=f"st2{h}{b}")
                    stats_mms(stat2[0:1, 0, :], sq_w)
                    nrow = row("nr")
                    nc.scalar.activation(
                        nrow[:], stat2[0:1, 0, :], AF.Sqrt, bias=eps_n2[:]
                    )
                    rr = row("rr")
                    nc.vector.reciprocal_approx_fast(rr[:], nrow[:])
                    rb = bcast128(rr[:], "rb")
                    for u in range(2):
                        nc.vector.tensor_tensor(
                            obt[:, h * DC + 2 * u : h * DC + 2 * u + 2, :],
                            ounits[u][:],
                            rb[:, None, :].to_broadcast((P, 2, 512)),
                            ALU.mult,
                        )

                for gb in range(NGB):
                    g0 = gb * 512
                    units = [punit(), punit()]
                    bo_sl = bslp.tile([P, 512], F32, tag="bo", name=f"bo{gb}{b}")
                    nc.sync.dma_start(bo_sl[:], _bcast_ap(bo_d, g0, 512))
                    for kb in range(8):
                        _ctr[0] += 1
                        blk = wblkp.tile([P, 4, 512], BF16, tag="wblk", name=f"wo{_ctr[0]}")
                        nc.sync.dma_start(
                            blk[:],
                            wot_d[4 * kb : 4 * kb + 4, :, g0 : g0 + 512].rearrange(
                                "k p f -> p k f"
                            ),
                        )
                        for j in range(4):
                            ko = 4 * kb + j
                            for t_ in range(TC):
                                nc.tensor.matmul(
                                    units[t_ // 2][:, t_ % 2, :],
                                    obt[:, ko, t_ * P : (t_ + 1) * P],
                                    blk[:, j, :],
                                    start=(ko == 0),
                                    stop=(ko == KO - 1),
                                )
                    for t_ in range(TC):
                        y_sb = youtp.tile([P, 512], F32, tag="y", name=f"y{gb}{t_}{b}")
                        nc.vector.tensor_add(
                            y_sb[:], units[t_ // 2][:, t_ % 2, :], bo_sl[:]
                        )
                        nc.sync.dma_start(
                            y_d[b, t_ * P : (t_ + 1) * P, g0 : g0 + 512], y_sb[:]
                        )

    nc.compile()
    return nc


def _prep_general(x, Wq, bq, Wk, bk, Wv, bv, g_q, b_q, g_k, b_k, Wg, bg, Wo, bo):
    x = np.asarray(x, np.float32)
    scale = 1.0 / np.sqrt(HD)

    def center(W, bvec):
        W4 = np.asarray(W, np.float32).reshape(H, HD, E)
        Wc = W4 - W4.mean(axis=1, keepdims=True)
        b4 = np.asarray(bvec, np.float32).reshape(H, HD)
        bc = b4 - b4.mean(axis=1, keepdims=True)
        return Wc.reshape(E, E), bc.reshape(E)

    Wq_c, bq_c = center(Wq, bq)
    Wk_c, bk_c = center(Wk, bk)

    def to_kpf(W):
        return np.ascontiguousarray(
            np.asarray(W, np.float32).T.reshape(KO, P, E)
        ).astype(BF)

    shared = {
        "wqt": to_kpf(Wq_c),
        "wkt": to_kpf(Wk_c),
        "wvt": to_kpf(np.asarray(Wv, np.float32)),
        "wot": to_kpf(np.asarray(Wo, np.float32)),
        "wgt": np.ascontiguousarray(
            np.asarray(Wg, np.float32).T.reshape(TC, P, 2 * HD)
        ).astype(BF),
        "bqc": bq_c.reshape(KO, P).astype(np.float32),
        "bkc": bk_c.reshape(KO, P).astype(np.float32),
        "gq": (np.asarray(g_q, np.float32) * scale).reshape(DC, P),
        "bqn": (np.asarray(b_q, np.float32) * scale).reshape(DC, P),
        "gk": np.asarray(g_k, np.float32).reshape(DC, P),
        "bkn": np.asarray(b_k, np.float32).reshape(DC, P),
        "bgc": np.asarray(bg, np.float32).reshape(FC, P),
        "bv": np.asarray(bv, np.float32),
        "bo": np.asarray(bo, np.float32),
    }
    shared = {k: np.ascontiguousarray(v) for k, v in shared.items()}

    xt = np.ascontiguousarray(x.transpose(0, 2, 1)).reshape(B, KO, P, S).astype(BF)
    in_maps = []
    for c in range(N_CORES):
        m = dict(shared)
        m["xt"] = np.ascontiguousarray(xt[c * NB : (c + 1) * NB])
        in_maps.append(m)
    return in_maps


# =====================================================================

_NC_CACHE = {}


def _get_nc(fast: bool):
    key = "fast" if fast else "general"
    if key not in _NC_CACHE:
        _install_ntff_hook()
        _NC_CACHE[key] = _build_fast_program() if fast else _build_general_program()
    return _NC_CACHE[key]


def _is_fast_case(bq, bk, bv, g_q, b_q, g_k, b_k, bg, bo):
    zeros = all(
        np.all(np.asarray(a) == 0.0) for a in (bq, bk, bv, b_q, b_k, bg, bo)
    )
    ones = all(np.all(np.asarray(a) == 1.0) for a in (g_q, g_k))
    return zeros and ones


def _run(trace, **inputs):
    fast = _is_fast_case(
        inputs["bq"], inputs["bk"], inputs["bv"], inputs["g_q"], inputs["b_q"],
        inputs["g_k"], inputs["b_k"], inputs["bg"], inputs["bo"],
    )
    nc = _get_nc(fast)
    in_maps = _prep_fast(**inputs) if fast else _prep_general(**inputs)
    res = run_bass_kernel_spmd(nc, in_maps, list(range(N_CORES)), trace=trace)
    out = np.empty((B, S, E), np.float32)
    for c in range(N_CORES):
        out[c * NB : (c + 1) * NB] = res.results[c]["y"]
    return out, res


def kernel(**inputs) -> np.ndarray:
    out, _ = _run(False, **inputs)
    return out


def kernel_profiled(**inputs):
    """Like kernel() but with NTFF tracing; returns (out, BassKernelResults)."""
    return _run(True, **inputs)


# revision 9
# speedup vs baseline: 1.1522x; 1.0396x over previous
"""Trainium2 Bass kernel for nn_MultiHeadAttention_833223655722.

Data-parallel over batch (16 / 8 cores = 2 per core). All matmuls bf16 with
fp32 PSUM accumulation. LayerNorm mean folded into centered projection
weights; rstd factors folded into the scores consume (fast path).

v2: one-level Strassen on all four E x E matmuls (7/8 multiply count):
  - Q/K projections (weight-stationary, qT[f,t] output): token split halves
    the moving free dim -> N=256 matmuls (still full rate, LDW hidden).
  - V projection and out-projection run the same scheme "executed
    transposed" (data-stationary, moving weight combos) so the weight
    output dim stays on the free axis -> N=512 matmuls.
  - The seven x-side operands (host-precomputed, bf16) serve Q/K as moving
    operands and V as stationary operands (identical storage layout).
  - W-side combos precomputed on host; C-block assembly on DVE from PSUM.
LN/L2 stats: ACT square + DVE pair-sum tree + a single ones-matmul
(1 matmul per stat instead of 4).

The general path (nonzero biases / non-unit gains) keeps the v1 direct
implementation.
"""

import sys
import types

import numpy as np
import ml_dtypes

import concourse.bass as bass
import concourse.mybir as mybir
import concourse.tile as tile
from concourse import bacc, bass_isa, library_config
from concourse import bass_utils
from concourse.bass_utils import run_bass_kernel_spmd

# ---------------------------------------------------------------- constants
B, S, E, H = 16, 512, 4096, 8
HD = E // H            # 512 (== S)
N_CORES = 8
NB = B // N_CORES      # 2 batches per core
P = 128
KO = E // P            # 32 contraction chunks over E
KO2 = KO // 2          # 16 chunks per e-half
TC = S // P            # 4 token chunks
DC = HD // P           # 4 head-dim chunks
FC = 2 * HD // P       # 8 GeGLU chunks
NGB = E // 512         # 8 out-proj column blocks
EH = E // 2            # 2048
SH = S // 2            # 256
LN_EPS = 1e-5
NORM_EPS = 1e-12

F32 = mybir.dt.float32
BF16 = mybir.dt.bfloat16
BF = ml_dtypes.bfloat16
AF = mybir.ActivationFunctionType
ALU = mybir.AluOpType


def _install_ntff_hook():
    """Register the NTFF profile hook missing from this image's antenv."""
    try:
        import antenv
        from trn_agent_boot.trn_boot import _ntff_profile_via_ctypes

        if "antenv.axon_hooks" in sys.modules:
            return
        hook = _ntff_profile_via_ctypes("/opt/axon/libaxon_pjrt.so")
        mod = types.ModuleType("antenv.axon_hooks")
        mod.get_axon_ntff_profile_hook = lambda: hook
        mod.set_axon_ntff_profile_hook = lambda h: None
        sys.modules["antenv.axon_hooks"] = mod
        antenv.axon_hooks = mod
        bass_utils.upload_artifacts = lambda tmpdir: tmpdir
    except Exception:
        pass


# =====================================================================
# fast path (all biases zero, gains one): Strassen build
# =====================================================================

def _build_fast_program():
    nc = bacc.Bacc("TRN2", target_bir_lowering=False, debug=False, num_devices=N_CORES)

    def dm(name, shape, dt, **kw):
        return nc.dram_tensor(name, shape, dt, **kw).ap()

    # seven x-side operands per batch, partition-major [op, P(e), ko(e), t]
    xs_d = dm("xs", [NB, 7, P, KO2, SH], BF16, kind="ExternalInput")
    # Q/K W-side operands, slab-major [op, mc, P(e), ko(e), f(128)]
    wqst_d = dm("wqst", [7, 16, P, KO2, P], BF16, kind="ExternalInput")
    wkst_d = dm("wkst", [7, 16, P, KO2, P], BF16, kind="ExternalInput")
    # V/O W-side operands, slab-major [op, gcol, P(e), ko(e), out(512)]
    wvst_d = dm("wvst", [7, 4, P, KO2, 512], BF16, kind="ExternalInput")
    wost_d = dm("wost", [7, 4, P, KO2, 512], BF16, kind="ExternalInput")
    wgt_d = dm("wgt", [TC, P, 2 * HD], BF16, kind="ExternalInput")
    y_d = dm("y", [NB, S, E], F32, kind="ExternalOutput")
    rksc_d = dm("rksc", [NB * H, 512], F32)

    with tile.TileContext(nc) as tc:
        with (
            tc.tile_pool(name="singles", bufs=1) as singles,
            tc.tile_pool(name="obtp", bufs=1) as obtp,
            tc.tile_pool(name="xsp", bufs=1) as xsp,
            tc.tile_pool(name="rows", bufs=2) as rowsp,
            tc.tile_pool(name="cols", bufs=4) as colsp,
            tc.tile_pool(name="ps", bufs=4, space="PSUM") as psp,
        ):
            nc.gpsimd.load_library(library_config.attn)

            eps_qf = singles.tile([P, 1], F32)
            nc.vector.memset(eps_qf[:], float(HD * LN_EPS))
            eps_ln = singles.tile([P, 1], F32)
            nc.vector.memset(eps_ln[:], float(LN_EPS))
            eps_n2 = singles.tile([P, 1], F32)
            nc.vector.memset(eps_n2[:], float(NORM_EPS**2))
            wgt_sb = singles.tile([P, TC, 2 * HD], BF16)
            nc.sync.dma_start(wgt_sb[:], wgt_d.rearrange("t p f -> p t f"))

            _ctr = [0]

            def punit():
                _ctr[0] += 1
                return psp.tile([P, 2, 512], F32, tag="u", name=f"u{_ctr[0]}")

            def fullrow(tag):
                _ctr[0] += 1
                return rowsp.tile([P, 512], F32, tag=tag, name=f"{tag}{_ctr[0]}", bufs=2)

            for b in range(NB):
                obt = obtp.tile([P, KO, S], BF16, tag="obt", name=f"obt{b}")
                xs = xsp.tile([P, 7, KO2, SH], BF16, tag="xs", name=f"xs{b}")
                for op in range(7):
                    nc.gpsimd.dma_start(xs[:, op], xs_d[b, op])

                with (
                    tc.tile_pool(name=f"wsl{b}", bufs=3) as wslp,
                    tc.tile_pool(name=f"wvsl{b}", bufs=2) as wvslp,
                    tc.tile_pool(name=f"act{b}", bufs=2) as actp,
                    tc.tile_pool(name=f"act3{b}", bufs=2) as actp3,
                    tc.tile_pool(name=f"sqp{b}", bufs=1) as sqp,
                    tc.tile_pool(name=f"ctmp{b}", bufs=1) as ctmpp,
                    tc.tile_pool(name=f"vtmp{b}", bufs=1) as vtmpp,
                    tc.tile_pool(name=f"stt{b}", bufs=2) as sttp,
                ):
                    def wslab_qk(w_dram, op, mc):
                        _ctr[0] += 1
                        t = wslp.tile([P, KO2, P], BF16, tag="wsl", name=f"w{_ctr[0]}")
                        eng = nc.sync if op % 2 == 0 else nc.scalar
                        eng.dma_start(t[:], w_dram[op, mc])
                        return t

                    def wslab_mov(w_dram, op, half, gcol):
                        """[P, 8, 512] slab: ko-half of a moving W operand."""
                        _ctr[0] += 1
                        t = wvslp.tile([P, KO2 // 2, 512], BF16, tag="wvsl", name=f"wv{_ctr[0]}")
                        eng = nc.sync if op % 2 == 0 else nc.scalar
                        eng.dma_start(t[:], w_dram[op, gcol, :, half * 8 : half * 8 + 8, :])
                        return t

                    def stat_full(src, name):
                        """src [P,4,512] bf16 -> sum of squares all-reduced across
                        partitions -> [P,512] f32 (same value in every partition)."""
                        _ctr[0] += 1
                        sq = sqp.tile([P, DC, S], BF16, tag="sq", name=f"sq{name}{_ctr[0]}")
                        nc.scalar.activation(sq[:], src[:], AF.Square)
                        t0 = sttp.tile([P, 512], F32, tag="st0", name=f"st0{_ctr[0]}", bufs=1)
                        sbf = sttp.tile([P, 512], BF16, tag="stb", name=f"stb{_ctr[0]}")
                        nc.vector.tensor_tensor(t0[:], sq[:, 0, :], sq[:, 1, :], ALU.add)
                        nc.vector.tensor_tensor(t0[:], t0[:], sq[:, 2, :], ALU.add)
                        nc.vector.tensor_tensor(sbf[:], t0[:], sq[:, 3, :], ALU.add)
                        r = sttp.tile([P, 512], F32, tag="str", name=f"str{_ctr[0]}")
                        nc.gpsimd.partition_all_reduce(
                            r[:], sbf[:], P, bass_isa.ReduceOp.add
                        )
                        return r

                    # C-combo helper for Q/K quarters (one PSUM read per op)
                    def qk_combine(quarters, out_lo, out_hi, dc):
                        """quarters: list of 7 psum APs [P, 256] (M1..M7)."""
                        M = quarters
                        t = ctmpp.tile([P, SH], F32, tag="ct", name=f"ct{_ctr[0]}a")
                        _ctr[0] += 1
                        nc.vector.tensor_copy(t[:], M[0])
                        nc.vector.tensor_tensor(t[:], t[:], M[3], ALU.add)
                        nc.vector.tensor_tensor(t[:], t[:], M[4], ALU.subtract)
                        nc.vector.tensor_tensor(out_lo[:, dc, 0:SH], t[:], M[6], ALU.add)
                        t2 = ctmpp.tile([P, SH], F32, tag="ct2", name=f"ct{_ctr[0]}b")
                        _ctr[0] += 1
                        nc.vector.tensor_copy(t2[:], M[2])
                        nc.vector.tensor_tensor(out_lo[:, dc, SH:S], t2[:], M[4], ALU.add)
                        t3 = ctmpp.tile([P, SH], F32, tag="ct3", name=f"ct{_ctr[0]}c")
                        _ctr[0] += 1
                        nc.vector.tensor_copy(t3[:], M[1])
                        nc.vector.tensor_tensor(out_hi[:, dc, 0:SH], t3[:], M[3], ALU.add)
                        t4 = ctmpp.tile([P, SH], F32, tag="ct4", name=f"ct{_ctr[0]}d")
                        _ctr[0] += 1
                        nc.vector.tensor_copy(t4[:], M[0])
                        nc.vector.tensor_tensor(t4[:], t4[:], M[1], ALU.subtract)
                        nc.vector.tensor_tensor(t4[:], t4[:], M[2], ALU.add)
                        nc.vector.tensor_tensor(out_hi[:, dc, SH:S], t4[:], M[5], ALU.add)

                    def qk_proj(w_dram, g, out_lo, out_hi, name, inject=None):
                        """Strassen products for f-rows of heads (g, 4+g)."""
                        for mc in range(4 * g, 4 * g + 4):
                            if inject and (mc - 4 * g) in inject:
                                inject[mc - 4 * g]()
                            ua = punit()
                            ub = punit()
                            quarters = [
                                ua[:, 0, 0:SH], ua[:, 0, SH:512],
                                ua[:, 1, 0:SH], ua[:, 1, SH:512],
                                ub[:, 0, 0:SH], ub[:, 0, SH:512],
                                ub[:, 1, 0:SH],
                            ]
                            for op in range(7):
                                slab = wslab_qk(w_dram, op, mc)
                                for ko in range(KO2):
                                    nc.tensor.matmul(
                                        quarters[op],
                                        slab[:, ko, :],
                                        xs[:, op, ko, :],
                                        start=(ko == 0),
                                        stop=(ko == KO2 - 1),
                                    )
                            qk_combine(quarters, out_lo, out_hi, mc - 4 * g)

                    def v_proj(g, vc_lo, vc_hi, inject=None):
                        """Transposed-scheme products, d-columns of heads (g, 4+g)."""
                        t11 = vtmpp.tile([P, 2, 512], F32, tag="v11", name=f"v11{b}{g}")
                        t12 = vtmpp.tile([P, 2, 512], BF16, tag="v12", name=f"v12{b}{g}")
                        t21 = vtmpp.tile([P, 2, 512], BF16, tag="v21", name=f"v21{b}{g}")
                        t22 = vtmpp.tile([P, 2, 512], F32, tag="v22", name=f"v22{b}{g}")
                        for op in range(7):
                            if inject and op in inject:
                                inject[op]()
                            u = punit()
                            for half in range(2):
                                slab = wslab_mov(wvst_d, op, half, g)
                                for kk in range(KO2 // 2):
                                    ko = half * 8 + kk
                                    for tcc in range(2):
                                        nc.tensor.matmul(
                                            u[:, tcc, :],
                                            xs[:, op, ko, tcc * P : (tcc + 1) * P],
                                            slab[:, kk, :],
                                            start=(ko == 0),
                                            stop=(ko == KO2 - 1),
                                        )
                            # greedy C accumulation (P1..P7 = op 0..6)
                            if op == 0:
                                nc.vector.tensor_copy(t11[:], u[:])
                                nc.vector.tensor_copy(t22[:], u[:])
                            elif op == 1:
                                nc.vector.tensor_copy(t12[:], u[:])
                                nc.vector.tensor_tensor(t22[:], t22[:], u[:], ALU.subtract)
                            elif op == 2:
                                nc.vector.tensor_copy(t21[:], u[:])
                                nc.vector.tensor_tensor(t22[:], t22[:], u[:], ALU.add)
                            elif op == 3:
                                nc.vector.tensor_tensor(t11[:], t11[:], u[:], ALU.add)
                                nc.vector.tensor_tensor(vc_hi[:, 0:2, :], t12[:], u[:], ALU.add)
                            elif op == 4:
                                nc.vector.tensor_tensor(t11[:], t11[:], u[:], ALU.subtract)
                                nc.vector.tensor_tensor(vc_lo[:, 2:4, :], t21[:], u[:], ALU.add)
                            elif op == 5:
                                nc.vector.tensor_tensor(vc_hi[:, 2:4, :], t22[:], u[:], ALU.add)
                            else:
                                nc.vector.tensor_tensor(vc_lo[:, 0:2, :], t11[:], u[:], ALU.add)

                    # ---------------- attention for one head ----------------
                    def attention(h, qc, kc, vc, rqb, rk_cols):
                        sunits = [punit(), punit()]
                        for t_ in range(TC):
                            for dc in range(DC):
                                nc.tensor.matmul(
                                    sunits[t_ // 2][:, t_ % 2, :],
                                    kc[:, dc, t_ * P : (t_ + 1) * P],
                                    qc[:, dc, :],
                                    start=(dc == 0),
                                    stop=(dc == DC - 1),
                                )
                        sc = actp3.tile([P, TC, S], BF16, tag="sc", name=f"sc{h}{b}")
                        for t_ in range(TC):
                            nc.vector.scalar_tensor_tensor(
                                sc[:, t_, :],
                                sunits[t_ // 2][:, t_ % 2, :],
                                rk_cols[:, t_ : t_ + 1],
                                rqb[:],
                                ALU.mult,
                                ALU.mult,
                            )
                        gunits = [punit(), punit()]
                        for i in range(DC):
                            fc = DC + i
                            for t_ in range(TC):
                                nc.tensor.matmul(
                                    gunits[i // 2][:, i % 2, :],
                                    wgt_sb[:, t_, fc * P : (fc + 1) * P],
                                    sc[:, t_, :],
                                    start=(t_ == 0),
                                    stop=(t_ == TC - 1),
                                )
                        vunits2 = [punit(), punit()]
                        for i in range(DC):
                            for t_ in range(TC):
                                nc.tensor.matmul(
                                    vunits2[i // 2][:, i % 2, :],
                                    wgt_sb[:, t_, i * P : (i + 1) * P],
                                    sc[:, t_, :],
                                    start=(t_ == 0),
                                    stop=(t_ == TC - 1),
                                )
                        gel = actp3.tile([P, DC, S], BF16, tag="gel", name=f"gel{h}{b}", bufs=1)
                        for i in range(DC):
                            nc.scalar.activation(
                                gel[:, i, :],
                                gunits[i // 2][:, i % 2, :],
                                AF.Gelu,
                                bias=0.0,
                            )
                        wv = actp3.tile([P, DC, S], BF16, tag="wv", name=f"wv{h}{b}", bufs=1)
                        for u in range(2):
                            nc.vector.tensor_copy(
                                wv[:, 2 * u : 2 * u + 2, :], vunits2[u][:]
                            )
                            nc.vector.tensor_mul(
                                wv[:, 2 * u : 2 * u + 2, :],
                                wv[:, 2 * u : 2 * u + 2, :],
                                gel[:, 2 * u : 2 * u + 2, :],
                            )
                        # L2 stats via gpsimd all-reduce (no PE, no broadcast)
                        r_w = stat_full(wv, f"w{h}")
                        # out matmuls (t-major)
                        ounits = [punit(), punit()]
                        for t_ in range(TC):
                            for dc in range(DC):
                                nc.tensor.matmul(
                                    ounits[dc // 2][:, dc % 2, :],
                                    vc[:, t_, dc * P : (dc + 1) * P],
                                    wv[:, t_, :],
                                    start=(t_ == 0),
                                    stop=(t_ == TC - 1),
                                )
                        nrow = fullrow("sd")
                        nc.scalar.activation(nrow[:], r_w[:], AF.Sqrt, bias=eps_n2[:])
                        rb = fullrow("rq")
                        nc.vector.reciprocal_approx_fast(rb[:], nrow[:])
                        for u in range(2):
                            nc.vector.tensor_tensor(
                                obt[:, h * DC + 2 * u : h * DC + 2 * u + 2, :],
                                ounits[u][:],
                                rb[:, None, :].to_broadcast((P, 2, 512)),
                                ALU.mult,
                            )

                    # =============== head-pair groups ===============
                    for g in range(4):
                        hl, hh = g, 4 + g

                        qc_lo = actp.tile([P, DC, S], BF16, tag="qc", name=f"qc{hl}{b}")
                        qc_hi = actp.tile([P, DC, S], BF16, tag="qc", name=f"qc{hh}{b}")
                        qk_proj(wqst_d, g, qc_lo, qc_hi, "q")

                        kc_lo = actp.tile([P, DC, S], BF16, tag="kc", name=f"kc{hl}{b}")
                        kc_hi = actp.tile([P, DC, S], BF16, tag="kc", name=f"kc{hh}{b}")
                        # q stats: square+tree+all-reduce, rows off the PE path
                        r_ql = stat_full(qc_lo, f"ql{g}")
                        r_qh = stat_full(qc_hi, f"qh{g}")
                        rqbs = []

                        def q_rows():
                            for r_q in (r_ql, r_qh):
                                sd_q = fullrow("sd")
                                nc.scalar.activation(
                                    sd_q[:], r_q[:], AF.Sqrt, bias=eps_qf[:]
                                )
                                rqb = fullrow("rq")
                                nc.vector.reciprocal_approx_fast(rqb[:], sd_q[:])
                                rqbs.append(rqb)

                        qk_proj(wkst_d, g, kc_lo, kc_hi, "k", inject={2: q_rows})

                        vc_lo = actp.tile([P, TC, HD], BF16, tag="vc", name=f"vc{hl}{b}")
                        vc_hi = actp.tile([P, TC, HD], BF16, tag="vc", name=f"vc{hh}{b}")
                        # k stats
                        r_kl = stat_full(kc_lo, f"kl{g}")
                        r_kh = stat_full(kc_hi, f"kh{g}")
                        rkcs = []

                        def k_rows():
                            for r_k, h in ((r_kl, hl), (r_kh, hh)):
                                sd_k = fullrow("sdk")
                                nc.scalar.activation(
                                    sd_k[:], r_k[:], AF.Sqrt,
                                    bias=eps_ln[:], scale=float(1.0 / HD),
                                )
                                idx = b * H + h
                                nc.sync.dma_start(rksc_d[idx : idx + 1, :], sd_k[0:1, :])
                                sd_cols = colsp.tile([P, TC], F32, tag="cols", name=f"sdc{h}{b}")
                                nc.sync.dma_start(
                                    sd_cols[:], rksc_d[idx].rearrange("(c p) -> p c", p=P)
                                )
                                rk_cols = colsp.tile([P, TC], F32, tag="cols", name=f"rkc{h}{b}")
                                nc.vector.reciprocal_approx_fast(rk_cols[:], sd_cols[:])
                                rkcs.append(rk_cols)

                        v_proj(g, vc_lo, vc_hi, inject={3: k_rows})

                        attention(hl, qc_lo, kc_lo, vc_lo, rqbs[0], rkcs[0])
                        attention(hh, qc_hi, kc_hi, vc_hi, rqbs[1], rkcs[1])

                # ---------------- out-projection (Strassen) ----------------
                with (
                    tc.tile_pool(name=f"ocp{b}", bufs=1) as ocp,
                    tc.tile_pool(name=f"wosl{b}", bufs=3) as woslp,
                    tc.tile_pool(name=f"yac{b}", bufs=1) as yacp,
                ):
                    # obtT blocks: O11=obt[:,0:16,0:256] O12=[...,256:512]
                    #              O21=obt[:,16:32,0:256] O22=[...,16:32,256:512]
                    O11 = obt[:, 0:KO2, 0:SH]
                    O12 = obt[:, 0:KO2, SH:S]
                    O21 = obt[:, KO2:KO, 0:SH]
                    O22 = obt[:, KO2:KO, SH:S]

                    def occombo(a, bb, alu, name):
                        _ctr[0] += 1
                        t = ocp.tile([P, KO2, SH], BF16, tag=name, name=f"{name}{_ctr[0]}")
                        nc.vector.tensor_tensor(t[:], a, bb, alu)
                        return t

                    oc1 = occombo(O11, O22, ALU.add, "oc1")
                    oc3 = occombo(O12, O22, ALU.subtract, "oc3")
                    oc4 = occombo(O21, O11, ALU.subtract, "oc4")
                    oc6 = occombo(O11, O12, ALU.add, "oc6")
                    oc7 = occombo(O21, O22, ALU.add, "oc7")
                    # stationary operand per product (M1..M7)
                    ostat = [oc1[:], O11, oc3[:], oc4[:], O22, oc6[:], oc7[:]]

                    for gc in range(4):
                        ty11 = yacp.tile([P, 2, 512], F32, tag="y11", name=f"y11{b}{gc}")
                        ty12 = yacp.tile([P, 2, 512], F32, tag="y12", name=f"y12{b}{gc}")
                        ty21 = yacp.tile([P, 2, 512], F32, tag="y21", name=f"y21{b}{gc}")
                        ty22 = yacp.tile([P, 2, 512], F32, tag="y22", name=f"y22{b}{gc}")
                        # raw-operand products first: their MMs start while the
                        # DVE is still building the oc combos
                        for op in (1, 4, 0, 2, 3, 5, 6):
                            u = punit()
                            for half in range(2):
                                slab = woslp.tile(
                                    [P, KO2 // 2, 512], BF16, tag="wosl",
                                    name=f"wo{b}{gc}{op}{half}",
                                )
                                eng = nc.sync if op % 2 == 0 else nc.scalar
                                eng.dma_start(
                                    slab[:],
                                    wost_d[op, gc, :, half * 8 : half * 8 + 8, :],
                                )
                                for kk in range(KO2 // 2):
                                    ko = half * 8 + kk
                                    for tcc in range(2):
                                        nc.tensor.matmul(
                                            u[:, tcc, :],
                                            ostat[op][:, ko, tcc * P : (tcc + 1) * P],
                                            slab[:, kk, :],
                                            start=(ko == 0),
                                            stop=(ko == KO2 - 1),
                                        )
                            if op == 1:      # P2
                                nc.vector.tensor_copy(ty12[:], u[:])
                                nc.vector.tensor_scalar(ty22[:], u[:], -1.0, None, ALU.mult)
                            elif op == 4:    # P5
                                nc.vector.tensor_copy(ty21[:], u[:])
                                nc.vector.tensor_scalar(ty11[:], u[:], -1.0, None, ALU.mult)
                            elif op == 0:    # P1
                                nc.vector.tensor_tensor(ty11[:], ty11[:], u[:], ALU.add)
                                nc.vector.tensor_tensor(ty22[:], ty22[:], u[:], ALU.add)
                            elif op == 2:    # P3
                                nc.vector.tensor_tensor(ty21[:], ty21[:], u[:], ALU.add)
                                nc.vector.tensor_tensor(ty22[:], ty22[:], u[:], ALU.add)
                            elif op == 3:    # P4
                                nc.vector.tensor_tensor(ty11[:], ty11[:], u[:], ALU.add)
                                nc.vector.tensor_tensor(ty12[:], ty12[:], u[:], ALU.add)
                            elif op == 5:    # P6
                                nc.vector.tensor_tensor(ty22[:], ty22[:], u[:], ALU.add)
                            else:            # P7
                                nc.vector.tensor_tensor(ty11[:], ty11[:], u[:], ALU.add)
                        # y blocks: 11=[t0:256,g0half0] 12=[t0:256,half1]
                        #           21=[t256:512,half0] 22=[t256:512,half1]
                        g0a = gc * 512
                        g0b = EH + gc * 512
                        nc.sync.dma_start(
                            y_d[b, 0:SH, g0a : g0a + 512].rearrange("(j p) g -> p j g", p=P),
                            ty11[:],
                        )
                        nc.sync.dma_start(
                            y_d[b, 0:SH, g0b : g0b + 512].rearrange("(j p) g -> p j g", p=P),
                            ty12[:],
                        )
                        nc.sync.dma_start(
                            y_d[b, SH:S, g0a : g0a + 512].rearrange("(j p) g -> p j g", p=P),
                            ty21[:],
                        )
                        nc.sync.dma_start(
                            y_d[b, SH:S, g0b : g0b + 512].rearrange("(j p) g -> p j g", p=P),
                            ty22[:],
                        )

    nc.compile()
    return nc


def _prep_fast(x, Wq, bq, Wk, bk, Wv, bv, g_q, b_q, g_k, b_k, Wg, bg, Wo, bo):
    x = np.asarray(x, np.float32)

    def center(W):
        W4 = np.asarray(W, np.float32).reshape(H, HD, E)
        Wc = W4 - W4.mean(axis=1, keepdims=True)
        return Wc.reshape(E, E)

    def strassen_ops(G, blk):
        """Standard A-side patterns of G [out, e]; slab-major
        [7, EH//blk, P(e), KO2, blk] so each slab is contiguous per partition."""
        G11, G12 = G[:EH, :EH], G[:EH, EH:]
        G21, G22 = G[EH:, :EH], G[EH:, EH:]
        ops = [G11 + G22, G21 + G22, G11, G22, G11 + G12, G21 - G11, G12 - G22]
        nblk = EH // blk
        out = np.empty((7, nblk, P, KO2, blk), BF)
        for i, op in enumerate(ops):
            # op.T: [e 2048, out 2048] -> [KO2, P, nblk, blk] -> (nblk, P, KO2, blk)
            t = np.ascontiguousarray(op.T).reshape(KO2, P, nblk, blk)
            out[i] = t.transpose(2, 1, 0, 3).astype(BF)
        return out

    shared = {
        "wqst": strassen_ops(center(Wq), P),
        "wkst": strassen_ops(center(Wk), P),
        "wvst": strassen_ops(np.asarray(Wv, np.float32), 512),
        "wost": strassen_ops(np.asarray(Wo, np.float32), 512),
        "wgt": np.ascontiguousarray(
            np.asarray(Wg, np.float32).T.reshape(TC, P, 2 * HD)
        ).astype(BF),
    }

    in_maps = []
    for c in range(N_CORES):
        m = dict(shared)
        xsl = np.empty((NB, 7, P, KO2, SH), BF)
        for bi in range(NB):
            xT = np.ascontiguousarray(x[c * NB + bi].T)  # [E, S]
            B11, B12 = xT[:EH, :SH], xT[:EH, SH:]
            B21, B22 = xT[EH:, :SH], xT[EH:, SH:]
            ops = [B11 + B22, B11, B12 - B22, B21 - B11, B22, B11 + B12, B21 + B22]
            for i, op in enumerate(ops):
                # [2048e, SH] -> [KO2, P, SH] -> [P, KO2, SH]
                xsl[bi, i] = (
                    np.ascontiguousarray(op).reshape(KO2, P, SH)
                    .transpose(1, 0, 2).astype(BF)
                )
        m["xs"] = xsl
        in_maps.append(m)
    return in_maps


# =====================================================================
# general path: v1 direct implementation (biases / gains arbitrary)
# =====================================================================

def _bcast_ap(dram_ap, offset, n):
    return bass.AP(
        tensor=dram_ap.tensor, offset=dram_ap.offset + offset, ap=[[0, P], [1, n]]
    )


def _build_general_program():
    fast = False
    nc = bacc.Bacc("TRN2", target_bir_lowering=False, debug=False, num_devices=N_CORES)

    def dm(name, shape, dt, **kw):
        return nc.dram_tensor(name, shape, dt, **kw).ap()

    xt_d = dm("xt", [NB, KO, P, S], BF16, kind="ExternalInput")
    wqt_d = dm("wqt", [KO, P, E], BF16, kind="ExternalInput")
    wkt_d = dm("wkt", [KO, P, E], BF16, kind="ExternalInput")
    wvt_d = dm("wvt", [KO, P, E], BF16, kind="ExternalInput")
    wgt_d = dm("wgt", [TC, P, 2 * HD], BF16, kind="ExternalInput")
    wot_d = dm("wot", [KO, P, E], BF16, kind="ExternalInput")
    bqc_d = dm("bqc", [KO, P], F32, kind="ExternalInput")
    bkc_d = dm("bkc", [KO, P], F32, kind="ExternalInput")
    gq_d = dm("gq", [DC, P], F32, kind="ExternalInput")
    bqn_d = dm("bqn", [DC, P], F32, kind="ExternalInput")
    gk_d = dm("gk", [DC, P], F32, kind="ExternalInput")
    bkn_d = dm("bkn", [DC, P], F32, kind="ExternalInput")
    bgc_d = dm("bgc", [FC, P], F32, kind="ExternalInput")
    bv_d = dm("bv", [E], F32, kind="ExternalInput")
    bo_d = dm("bo", [E], F32, kind="ExternalInput")
    y_d = dm("y", [NB, S, E], F32, kind="ExternalOutput")
    rksc_d = dm("rksc", [NB * H, 512], F32)

    with tile.TileContext(nc) as tc:
        with (
            tc.tile_pool(name="singles", bufs=1) as singles,
            tc.tile_pool(name="xtp", bufs=1) as xtp,
            tc.tile_pool(name="obtp", bufs=1) as obtp,
            tc.tile_pool(name="wblk", bufs=6) as wblkp,
            tc.tile_pool(name="act", bufs=2) as actp,
            tc.tile_pool(name="act3", bufs=3) as actp3,
            tc.tile_pool(name="sqp", bufs=2) as sqp,
            tc.tile_pool(name="rows", bufs=6) as rowsp,
            tc.tile_pool(name="bc", bufs=3) as bcp,
            tc.tile_pool(name="bsl", bufs=2) as bslp,
            tc.tile_pool(name="cols", bufs=4) as colsp,
            tc.tile_pool(name="yout", bufs=2) as youtp,
            tc.tile_pool(name="ps", bufs=4, space="PSUM") as psp,
        ):
            nc.gpsimd.load_library(library_config.attn)

            ones_col = singles.tile([P, 1], BF16)
            nc.vector.memset(ones_col[:], 1.0)
            eps_qf = singles.tile([1, 1], F32)
            nc.vector.memset(eps_qf[:], float(HD * LN_EPS))
            eps_ln = singles.tile([1, 1], F32)
            nc.vector.memset(eps_ln[:], float(LN_EPS))
            eps_n2 = singles.tile([1, 1], F32)
            nc.vector.memset(eps_n2[:], float(NORM_EPS**2))
            wgt_sb = singles.tile([P, TC, 2 * HD], BF16)
            nc.sync.dma_start(wgt_sb[:], wgt_d.rearrange("t p f -> p t f"))

            def col_tile(dram, n):
                t = singles.tile([P, n], F32, name=f"ct_{dram.tensor.name}")
                nc.sync.dma_start(t[:], dram.rearrange("c p -> p c"))
                return t

            bqc_sb = col_tile(bqc_d, KO)
            bkc_sb = col_tile(bkc_d, KO)
            gq_sb = col_tile(gq_d, DC)
            bqn_sb = col_tile(bqn_d, DC)
            gk_sb = col_tile(gk_d, DC)
            bkn_sb = col_tile(bkn_d, DC)
            bgc_sb = col_tile(bgc_d, FC)

            _ctr = [0]

            def punit():
                _ctr[0] += 1
                return psp.tile([P, 2, 512], F32, tag="u", name=f"u{_ctr[0]}")

            def row(name):
                _ctr[0] += 1
                return rowsp.tile([1, 512], F32, tag="row", name=f"{name}{_ctr[0]}")

            def bcast128(row_ap, name):
                _ctr[0] += 1
                t = bcp.tile([P, 512], F32, tag="bc", name=f"{name}{_ctr[0]}")
                nc.gpsimd.partition_broadcast(t[:], row_ap)
                return t

            for b in range(NB):
                xt_sb = xtp.tile([P, KO, S], BF16, tag="xt")
                xt_splits = [(0, 1), (1, 4)] + [(4 * i, 4 * i + 4) for i in range(1, 8)]
                for lo, hi in xt_splits:
                    nc.gpsimd.dma_start(
                        xt_sb[:, lo:hi, :],
                        xt_d[b, lo:hi].rearrange("k p t -> p k t"),
                    )
                obt = obtp.tile([P, KO, S], BF16, tag="obt")

                for h in range(H):
                    f0 = h * HD

                    def wstream_blk(w_dram, kb, cols0, ncols):
                        _ctr[0] += 1
                        blk = wblkp.tile([P, 4, ncols], BF16, tag="wblk", name=f"w{_ctr[0]}")
                        nc.sync.dma_start(
                            blk[:],
                            w_dram[
                                4 * kb : 4 * kb + 4, :, cols0 : cols0 + ncols
                            ].rearrange("k p f -> p k f"),
                        )
                        return blk

                    def projT_mms(w_dram, units, kb):
                        blk = wstream_blk(w_dram, kb, f0, HD)
                        for j in range(4):
                            ko = 4 * kb + j
                            for dc in range(DC):
                                nc.tensor.matmul(
                                    units[dc // 2][:, dc % 2, :],
                                    blk[:, j, dc * P : (dc + 1) * P],
                                    xt_sb[:, ko, :],
                                    start=(ko == 0),
                                    stop=(ko == KO - 1),
                                )

                    def stats_mms(stat_slice, sq):
                        for dc in range(DC):
                            nc.tensor.matmul(
                                stat_slice,
                                ones_col[:],
                                sq[:, dc, :],
                                start=(dc == 0),
                                stop=(dc == DC - 1),
                            )

                    def consume_proj(units, bias_sb, name):
                        out_sb = actp.tile([P, DC, S], BF16, tag=name, name=f"{name}{h}{b}")
                        for dc in range(DC):
                            nc.vector.tensor_scalar(
                                out_sb[:, dc, :],
                                units[dc // 2][:, dc % 2, :],
                                bias_sb[:, h * DC + dc : h * DC + dc + 1],
                                None,
                                ALU.add,
                            )
                        sq = sqp.tile([P, DC, S], BF16, tag="sq", name=f"sq{name}{h}{b}")
                        nc.scalar.activation(sq[:], out_sb[:], AF.Square)
                        return out_sb, sq

                    qunits = [punit(), punit()]
                    for kb in range(4):
                        projT_mms(wqt_d, qunits, kb)
                    stat = psp.tile([1, 2, 512], F32, tag="u", name=f"st{h}{b}")
                    for kb in range(4, 8):
                        projT_mms(wqt_d, qunits, kb)
                    qc, sq_q = consume_proj(qunits, bqc_sb, "qc")

                    kunits = [punit(), punit()]
                    for kb in range(4):
                        projT_mms(wkt_d, kunits, kb)
                    stats_mms(stat[0:1, 0, :], sq_q)
                    for kb in range(4, 8):
                        projT_mms(wkt_d, kunits, kb)
                    kc, sq_k = consume_proj(kunits, bkc_sb, "kc")

                    sd_q = row("sdq")
                    nc.scalar.activation(
                        sd_q[:], stat[0:1, 0, :], AF.Sqrt,
                        bias=eps_ln[:], scale=float(1.0 / HD),
                    )
                    rq_row = row("rq")
                    nc.vector.reciprocal_approx_fast(rq_row[:], sd_q[:])
                    rqb = bcast128(rq_row[:], "rqb")

                    vunits = [punit(), punit()]
                    for kb in range(4):
                        blk = wstream_blk(wvt_d, kb, f0, HD)
                        for j in range(4):
                            ko = 4 * kb + j
                            for t_ in range(TC):
                                nc.tensor.matmul(
                                    vunits[t_ // 2][:, t_ % 2, :],
                                    xt_sb[:, ko, t_ * P : (t_ + 1) * P],
                                    blk[:, j, :],
                                    start=(ko == 0),
                                    stop=(ko == KO - 1),
                                )
                    stats_mms(stat[0:1, 1, :], sq_k)
                    for kb in range(4, 8):
                        blk = wstream_blk(wvt_d, kb, f0, HD)
                        for j in range(4):
                            ko = 4 * kb + j
                            for t_ in range(TC):
                                nc.tensor.matmul(
                                    vunits[t_ // 2][:, t_ % 2, :],
                                    xt_sb[:, ko, t_ * P : (t_ + 1) * P],
                                    blk[:, j, :],
                                    start=(ko == 0),
                                    stop=(ko == KO - 1),
                                )

                    sd_k = row("sdk")
                    nc.scalar.activation(
                        sd_k[:], stat[0:1, 1, :], AF.Sqrt,
                        bias=eps_ln[:], scale=float(1.0 / HD),
                    )
                    rk_row = row("rk")
                    nc.vector.reciprocal_approx_fast(rk_row[:], sd_k[:])
                    rkb = bcast128(rk_row[:], "rkb")
                    nc.vector.tensor_tensor(
                        kc[:], kc[:], rkb[:, None, :].to_broadcast((P, DC, S)), ALU.mult
                    )
                    for dc in range(DC):
                        nc.vector.tensor_scalar(
                            kc[:, dc, :],
                            kc[:, dc, :],
                            gk_sb[:, dc : dc + 1],
                            bkn_sb[:, dc : dc + 1],
                            ALU.mult,
                            ALU.add,
                        )
                    nc.vector.tensor_tensor(
                        qc[:], qc[:], rqb[:, None, :].to_broadcast((P, DC, S)), ALU.mult
                    )
                    for dc in range(DC):
                        nc.vector.tensor_scalar(
                            qc[:, dc, :],
                            qc[:, dc, :],
                            gq_sb[:, dc : dc + 1],
                            bqn_sb[:, dc : dc + 1],
                            ALU.mult,
                            ALU.add,
                        )

                    sunits = [punit(), punit()]
                    for t_ in range(TC):
                        for dc in range(DC):
                            nc.tensor.matmul(
                                sunits[t_ // 2][:, t_ % 2, :],
                                kc[:, dc, t_ * P : (t_ + 1) * P],
                                qc[:, dc, :],
                                start=(dc == 0),
                                stop=(dc == DC - 1),
                            )
                    sc = actp3.tile([P, TC, S], BF16, tag="sc", name=f"sc{h}{b}")
                    for u in range(2):
                        nc.vector.tensor_copy(sc[:, 2 * u : 2 * u + 2, :], sunits[u][:])

                    vc = actp.tile([P, TC, HD], BF16, tag="vc", name=f"vc{h}{b}")
                    bv_sl = bslp.tile([P, 512], F32, tag="bv", name=f"bv{h}{b}")
                    nc.sync.dma_start(bv_sl[:], _bcast_ap(bv_d, f0, 512))
                    for u in range(2):
                        nc.vector.tensor_tensor(
                            vc[:, 2 * u : 2 * u + 2, :],
                            vunits[u][:],
                            bv_sl[:, None, :].to_broadcast((P, 2, 512)),
                            ALU.add,
                        )

                    gunits = [punit(), punit()]
                    for i in range(DC):
                        fc = DC + i
                        for t_ in range(TC):
                            nc.tensor.matmul(
                                gunits[i // 2][:, i % 2, :],
                                wgt_sb[:, t_, fc * P : (fc + 1) * P],
                                sc[:, t_, :],
                                start=(t_ == 0),
                                stop=(t_ == TC - 1),
                            )
                    vunits2 = [punit(), punit()]
                    for i in range(DC):
                        for t_ in range(TC):
                            nc.tensor.matmul(
                                vunits2[i // 2][:, i % 2, :],
                                wgt_sb[:, t_, i * P : (i + 1) * P],
                                sc[:, t_, :],
                                start=(t_ == 0),
                                stop=(t_ == TC - 1),
                            )
                    gel = actp3.tile([P, DC, S], BF16, tag="gel", name=f"gel{h}{b}")
                    for i in range(DC):
                        nc.scalar.activation(
                            gel[:, i, :],
                            gunits[i // 2][:, i % 2, :],
                            AF.Gelu,
                            bias=bgc_sb[:, DC + i : DC + i + 1],
                        )
                    wv = actp3.tile([P, DC, S], BF16, tag="wv", name=f"wv{h}{b}")
                    for i in range(DC):
                        nc.vector.tensor_scalar(
                            wv[:, i, :],
                            vunits2[i // 2][:, i % 2, :],
                            bgc_sb[:, i : i + 1],
                            None,
                            ALU.add,
                        )
                    nc.vector.tensor_mul(wv[:], wv[:], gel[:])
                    sq_w = sqp.tile([P, DC, S], BF16, tag="sq", name=f"sqw{h}{b}")
                    nc.scalar.activation(sq_w[:], wv[:], AF.Square)

                    ounits = [punit(), punit()]
                    for t_ in range(TC):
                        for dc in range(DC):
                            nc.tensor.matmul(
                                ounits[dc // 2][:, dc % 2, :],
                                vc[:, t_, dc * P : (dc + 1) * P],
                                wv[:, t_, :],
                                start=(t_ == 0),
                                stop=(t_ == TC - 1),
                            )
                    stat2 = psp.tile([1, 2, 512], F32, tag="u", name=f"st2{h}{b}")
                    stats_mms(stat2[0:1, 0, :], sq_w)
                    nrow = row("nr")
                    nc.scalar.activation(
                        nrow[:], stat2[0:1, 0, :], AF.Sqrt, bias=eps_n2[:]
                    )
                    rr = row("rr")
                    nc.vector.reciprocal_approx_fast(rr[:], nrow[:])
                    rb = bcast128(rr[:], "rb")
                    for u in range(2):
                        nc.vector.tensor_tensor(
                            obt[:, h * DC + 2 * u : h * DC + 2 * u + 2, :],
                            ounits[u][:],
                            rb[:, None, :].to_broadcast((P, 2, 512)),
                            ALU.mult,
                        )

                for gb in range(NGB):
                    g0 = gb * 512
                    units = [punit(), punit()]
                    bo_sl = bslp.tile([P, 512], F32, tag="bo", name=f"bo{gb}{b}")
                    nc.sync.dma_start(bo_sl[:], _bcast_ap(bo_d, g0, 512))
                    for kb in range(8):
                        _ctr[0] += 1
                        blk = wblkp.tile([P, 4, 512], BF16, tag="wblk", name=f"wo{_ctr[0]}")
                        nc.sync.dma_start(
                            blk[:],
                            wot_d[4 * kb : 4 * kb + 4, :, g0 : g0 + 512].rearrange(
                                "k p f -> p k f"
                            ),
                        )
                        for j in range(4):
                            ko = 4 * kb + j
                            for t_ in range(TC):
                                nc.tensor.matmul(
                                    units[t_ // 2][:, t_ % 2, :],
                                    obt[:, ko, t_ * P : (t_ + 1) * P],
                                    blk[:, j, :],
                                    start=(ko == 0),
                                    stop=(ko == KO - 1),
                                )
                    for t_ in range(TC):
                        y_sb = youtp.tile([P, 512], F32, tag="y", name=f"y{gb}{t_}{b}")
                        nc.vector.tensor_add(
                            y_sb[:], units[t_ // 2][:, t_ % 2, :], bo_sl[:]
                        )
                        nc.sync.dma_start(
                            y_d[b, t_ * P : (t_ + 1) * P, g0 : g0 + 512], y_sb[:]
                        )

    nc.compile()
    return nc


def _prep_general(x, Wq, bq, Wk, bk, Wv, bv, g_q, b_q, g_k, b_k, Wg, bg, Wo, bo):
    x = np.asarray(x, np.float32)
    scale = 1.0 / np.sqrt(HD)

    def center(W, bvec):
        W4 = np.asarray(W, np.float32).reshape(H, HD, E)
        Wc = W4 - W4.mean(axis=1, keepdims=True)
        b4 = np.asarray(bvec, np.float32).reshape(H, HD)
        bc = b4 - b4.mean(axis=1, keepdims=True)
        return Wc.reshape(E, E), bc.reshape(E)

    Wq_c, bq_c = center(Wq, bq)
    Wk_c, bk_c = center(Wk, bk)

    def to_kpf(W):
        return np.ascontiguousarray(
            np.asarray(W, np.float32).T.reshape(KO, P, E)
        ).astype(BF)

    shared = {
        "wqt": to_kpf(Wq_c),
        "wkt": to_kpf(Wk_c),
        "wvt": to_kpf(np.asarray(Wv, np.float32)),
        "wot": to_kpf(np.asarray(Wo, np.float32)),
        "wgt": np.ascontiguousarray(
            np.asarray(Wg, np.float32).T.reshape(TC, P, 2 * HD)
        ).astype(BF),
        "bqc": bq_c.reshape(KO, P).astype(np.float32),
        "bkc": bk_c.reshape(KO, P).astype(np.float32),
        "gq": (np.asarray(g_q, np.float32) * scale).reshape(DC, P),
        "bqn": (np.asarray(b_q, np.float32) * scale).reshape(DC, P),
        "gk": np.asarray(g_k, np.float32).reshape(DC, P),
        "bkn": np.asarray(b_k, np.float32).reshape(DC, P),
        "bgc": np.asarray(bg, np.float32).reshape(FC, P),
        "bv": np.asarray(bv, np.float32),
        "bo": np.asarray(bo, np.float32),
    }
    shared = {k: np.ascontiguousarray(v) for k, v in shared.items()}

    xt = np.ascontiguousarray(x.transpose(0, 2, 1)).reshape(B, KO, P, S).astype(BF)
    in_maps = []
    for c in range(N_CORES):
        m = dict(shared)
        m["xt"] = np.ascontiguousarray(xt[c * NB : (c + 1) * NB])
        in_maps.append(m)
    return in_maps


# =====================================================================

_NC_CACHE = {}


def _get_nc(fast: bool):
    key = "fast" if fast else "general"
    if key not in _NC_CACHE:
        _install_ntff_hook()
        _NC_CACHE[key] = _build_fast_program() if fast else _build_general_program()
    return _NC_CACHE[key]


def _is_fast_case(bq, bk, bv, g_q, b_q, g_k, b_k, bg, bo):
    zeros = all(
        np.all(np.asarray(a) == 0.0) for a in (bq, bk, bv, b_q, b_k, bg, bo)
    )
    ones = all(np.all(np.asarray(a) == 1.0) for a in (g_q, g_k))
    return zeros and ones


def _run(trace, **inputs):
    fast = _is_fast_case(
        inputs["bq"], inputs["bk"], inputs["bv"], inputs["g_q"], inputs["b_q"],
        inputs["g_k"], inputs["b_k"], inputs["bg"], inputs["bo"],
    )
    nc = _get_nc(fast)
    in_maps = _prep_fast(**inputs) if fast else _prep_general(**inputs)
    res = run_bass_kernel_spmd(nc, in_maps, list(range(N_CORES)), trace=trace)
    out = np.empty((B, S, E), np.float32)
    for c in range(N_CORES):
        out[c * NB : (c + 1) * NB] = res.results[c]["y"]
    return out, res


def kernel(**inputs) -> np.ndarray:
    out, _ = _run(False, **inputs)
    return out


def kernel_profiled(**inputs):
    """Like kernel() but with NTFF tracing; returns (out, BassKernelResults)."""
    return _run(True, **inputs)
